# revision 23
# baseline (speedup 1.0000x reference)
"""DecoderTreeRNN Trainium2 kernel.

Strategy (8 NeuronCores):
  - Tree expansion: data-parallel over batch B (8 examples/core). GRU states
    are kept transposed [H, nodes] in bf16; each level computes
    ghT = WhhT.T @ hT (+ biases folded in via a K=1 ones-row matmul) with the
    weight tiles stationary on the PE, then gate math on ScalarE/VectorE.
    Children are concatenated [left | right]; the resulting leaf order is a
    bit-reversal permutation fixed up on the host.
  - Leaves are AllGathered (bf16) so every core holds all B*32 leaf states.
  - Output projection: tensor-parallel over vocab (4000 rows/core).
    W_outT shard stays resident in SBUF (bf16); logits accumulate in PSUM
    over 8 K-tiles (+ a K=1 bias matmul); exp+row-sum is fused on ScalarE
    (accum_out); per-group AllGathers reduce the softmax denominator across
    cores; final log-softmax subtract happens in place and streams out.
"""

import sys

for _p in ("/opt/trn_rl_repo",):
    if _p not in sys.path:
        sys.path.append(_p)

import numpy as np
import ml_dtypes

import concourse.bass as bass
from concourse import bacc, tile, mybir
from concourse import bass_utils
from contextlib import ExitStack

BF16 = mybir.dt.bfloat16
F32 = mybir.dt.float32
AF = mybir.ActivationFunctionType
ALU = mybir.AluOpType
BFNP = ml_dtypes.bfloat16
FP8 = mybir.dt.float8e4
FP8_AG = False  # fp8 all-gather: no faster (latency-bound), costs precision

N_CORES = 8
CW = 500  # vocab chunk width (<=512 fp32 psum bank)


def _build(B, H, V, DEPTH):
    KT = H // 128            # contraction tiles
    MT = 3 * KT              # output m-tiles per GRU side
    Bl = B // N_CORES        # examples per core
    L = 1 << DEPTH           # leaves per example
    NLOC = Bl * L            # local leaf count
    ROWS = B * L             # total leaf rows
    RT = ROWS // 128         # row tiles
    Vs = V // N_CORES        # vocab shard
    NCH = Vs // CW           # chunks per shard
    SG = min(4, KT)          # m-tiles per gate slab
    NSL = KT // SG           # slabs per gate
    assert B % N_CORES == 0 and H % 128 == 0 and V % N_CORES == 0
    assert Vs % CW == 0 and ROWS % 128 == 0 and RT <= 512
    assert SG * 128 <= 512  # psum slab fits one bank

    nc = bacc.Bacc("TRN2", target_bir_lowering=False, debug=False,
                   num_devices=N_CORES, dynamic_dma_scratch_size=2048)

    # ---------------- DRAM I/O ----------------
    encT = nc.dram_tensor("encT", [H, Bl], BF16, kind="ExternalInput")
    wt_d, wb_d, bih2_d = {}, {}, {}
    for s in "lr":
        wt_d[s] = nc.dram_tensor(f"wt_{s}", [H, 3 * H], BF16, kind="ExternalInput")
        wb_d[s] = nc.dram_tensor(f"wb_{s}", [1, 3 * H], BF16, kind="ExternalInput")
        bih2_d[s] = nc.dram_tensor(f"bih2_{s}", [128, KT, 128], BF16,
                                   kind="ExternalInput")
    woT_d = nc.dram_tensor("woT", [H, Vs], BF16, kind="ExternalInput")
    bo_d = nc.dram_tensor("bo", [128, Vs], F32, kind="ExternalInput")
    out_d = nc.dram_tensor("out", [ROWS, Vs], F32, kind="ExternalOutput")

    AGDT = FP8 if FP8_AG else BF16
    ag_leaves = nc.dram_tensor("ag_leaves", [N_CORES * H, NLOC], AGDT,
                               kind="Internal", addr_space="Shared")
    s_out_d = nc.dram_tensor("s_out", [128, RT], F32, kind="ExternalOutput")

    rg = [list(range(N_CORES))]

    with tile.TileContext(nc) as tc, ExitStack() as ctx:
        dram = ctx.enter_context(tc.tile_pool(name="dram", bufs=1, space="DRAM"))
        wproj = ctx.enter_context(tc.tile_pool(name="wproj", bufs=1))
        cpool = ctx.enter_context(tc.tile_pool(name="const", bufs=1))

        # projection weights: resident for the whole kernel. Tiles are
        # allocated up front but their DMAs are issued after the tree weight
        # DMAs (below) so the tree isn't starved of HBM bandwidth at start.
        wo_sb = [wproj.tile([128, Vs], BF16, tag=f"wo{k}", name=f"wo{k}")
                 for k in range(KT)]
        bo_sb = cpool.tile([128, Vs], F32, tag="bo")
        ones_sb = cpool.tile([1, 128], BF16, tag="ones")
        nc.vector.memset(ones_sb[:], 1.0)

        leaves_bounce = dram.tile([H, NLOC], AGDT, tag="lvb")

        # ---------------- tree expansion ----------------
        with nc.named_scope("tree"):
            with tc.tile_pool(name="wtree", bufs=1) as wtp, \
                 tc.tile_pool(name="state", bufs=2) as stp, \
                 tc.tile_pool(name="gates", bufs=2) as gp, \
                 tc.tile_pool(name="pstree", bufs=8, space="PSUM") as pst:
                # latency-critical small inputs go on the ACT HWDGE ring so
                # they aren't stuck behind the big weight loads (SP ring FIFO)
                cur = stp.tile([128, KT, Bl], BF16, tag="st")
                nc.scalar.dma_start(cur[:], encT.ap().rearrange("(k p) b -> p k b", k=KT))
                wt_sb, wb_sb, bih2_sb = {}, {}, {}
                for s in "lr":
                    wb_sb[s] = wtp.tile([1, 3 * H], BF16, tag=f"wb{s}", name=f"wb_sb_{s}")
                    nc.scalar.dma_start(wb_sb[s][:], wb_d[s].ap())
                    bih2_sb[s] = wtp.tile([128, KT, 128], BF16, tag=f"bi{s}", name=f"bih2_sb_{s}")
                    nc.scalar.dma_start(bih2_sb[s][:], bih2_d[s].ap())
                # weight loads in consumption order: side l, side r, then the
                # projection weights behind them (all FIFO on the SP ring)
                for s in "lr":
                    eng = nc.sync if s == "l" else nc.scalar
                    wt_sb[s] = []
                    for k in range(KT):
                        t = wtp.tile([128, 3 * H], BF16, tag=f"wt{s}{k}")
                        eng.dma_start(t[:], wt_d[s].ap()[128 * k:128 * (k + 1), :])
                        wt_sb[s].append(t)
                for k in range(KT):
                    nc.sync.dma_start(wo_sb[k][:], woT_d.ap()[128 * k:128 * (k + 1), :])
                nc.sync.dma_start(bo_sb[:], bo_d.ap())

                n = Bl
                for lvl in range(DEPTH):
                    nxt = stp.tile([128, KT, 2 * n], BF16, tag="st")
                    for si, s in enumerate("lr"):
                        for sl in range(NSL):
                            ko0 = sl * SG
                            ps = {}
                            for gi, mb in (("r", ko0), ("z", KT + ko0), ("g", 2 * KT + ko0)):
                                p = pst.tile([128, SG, n], F32, tag="ps")
                                for mj in range(SG):
                                    m = mb + mj
                                    for k in range(KT):
                                        nc.tensor.matmul(
                                            p[:, mj, :],
                                            wt_sb[s][k][:, 128 * m:128 * (m + 1)],
                                            cur[:, k, :n],
                                            start=(k == 0), stop=False)
                                    nc.tensor.matmul(
                                        p[:, mj, :],
                                        wb_sb[s][:, 128 * m:128 * (m + 1)],
                                        ones_sb[:, :n],
                                        start=False, stop=True)
                                ps[gi] = p
                            r_t = gp.tile([128, SG, n], F32, tag="r")
                            nc.scalar.activation(r_t[:], ps["r"][:], AF.Sigmoid)
                            z_t = gp.tile([128, SG, n], F32, tag="z")
                            nc.scalar.activation(z_t[:], ps["z"][:], AF.Sigmoid)
                            t_t = gp.tile([128, SG, n], F32, tag="t")
                            nc.vector.tensor_tensor(t_t[:], ps["g"][:], r_t[:], op=ALU.mult)
                            nc.vector.tensor_tensor(
                                t_t[:], t_t[:], bih2_sb[s][:, ko0:ko0 + SG, :n], op=ALU.add)
                            n_t = gp.tile([128, SG, n], F32, tag="n")
                            nc.scalar.activation(n_t[:], t_t[:], AF.Tanh)
                            u_t = gp.tile([128, SG, n], F32, tag="u")
                            nc.vector.scalar_tensor_tensor(
                                u_t[:], n_t[:], -1.0, cur[:, ko0:ko0 + SG, :n],
                                op0=ALU.mult, op1=ALU.add)  # u = h - n
                            nc.vector.tensor_tensor(u_t[:], u_t[:], z_t[:], op=ALU.mult)
                            nc.vector.tensor_tensor(
                                nxt[:, ko0:ko0 + SG, si * n:si * n + n],
                                u_t[:], n_t[:], op=ALU.add)
                    cur = nxt
                    n *= 2

                if FP8_AG:
                    lv8 = stp.tile([128, KT, NLOC], FP8, tag="lv8", name="lv8", bufs=1)
                    nc.vector.tensor_copy(lv8[:], cur[:])
                    src_t = lv8
                else:
                    src_t = cur
                for k in range(KT):
                    eng = nc.sync if k % 2 == 0 else nc.scalar
                    eng.dma_start(leaves_bounce[128 * k:128 * (k + 1), :],
                                  src_t[:, k, :])

        # ---------------- leaves all-gather ----------------
        with nc.named_scope("ag_leaves"):
            nc.gpsimd.collective_compute(
                "AllGather", ALU.bypass, replica_groups=rg,
                ins=[leaves_bounce.opt()], outs=[ag_leaves.ap()])

        # ---------------- projection + log-softmax ----------------
        with nc.named_scope("proj"):
            with tc.tile_pool(name="leaves", bufs=1) as lvp, \
                 tc.tile_pool(name="logits", bufs=3) as lgp, \
                 tc.tile_pool(name="scr", bufs=4) as scp, \
                 tc.tile_pool(name="stats", bufs=2) as sp2, \
                 tc.tile_pool(name="psproj", bufs=8, space="PSUM") as psp:
                lv_sb = []
                ag_view = ag_leaves.ap().rearrange("(c h) j -> h c j", c=N_CORES)
                for k in range(KT):
                    t = lvp.tile([128, N_CORES, NLOC], AGDT, tag=f"lv{k}")
                    eng = nc.sync if k % 2 == 0 else nc.scalar
                    eng.dma_start(t[:], ag_view[128 * k:128 * (k + 1)])
                    lv_sb.append(t)

                # unnormalized logits stream out as soon as each row tile is
                # done; the per-shard softmax denominators are returned as a
                # tiny second output and log(sum) is folded into the host-side
                # unshard pass.
                s_all = sp2.tile([128, RT], F32, tag="sall", name="s_all")
                for r in range(RT):
                    lg = lgp.tile([128, Vs], F32, tag="lg", name=f"lg{r}")
                    sp = sp2.tile([128, NCH], F32, tag="spart", name=f"sp{r}")
                    # k-outer so one stationary (leaves) tile serves all NCH
                    # chunks; each chunk accumulates in its own PSUM bank
                    pps = [psp.tile([128, CW], F32, tag="pp", name=f"pp{r}_{nch}")
                           for nch in range(NCH)]
                    for k in range(KT):
                        lhsT = lv_sb[k][:].rearrange("p c j -> p (c j)")[
                            :, 128 * r:128 * (r + 1)]
                        for nch in range(NCH):
                            nc.tensor.matmul(
                                pps[nch][:], lhsT,
                                wo_sb[k][:, CW * nch:CW * (nch + 1)],
                                start=(k == 0), stop=(k == KT - 1))
                    for nch in range(NCH):
                        # bias add fused into the PSUM->SBUF copy
                        nc.vector.tensor_tensor(
                            lg[:, CW * nch:CW * (nch + 1)], pps[nch][:],
                            bo_sb[:, CW * nch:CW * (nch + 1)],
                            op=ALU.add)
                        ex = scp.tile([128, CW], BF16, tag="exp",
                                      name=f"ex{r}_{nch}")
                        nc.scalar.activation(ex[:],
                                             lg[:, CW * nch:CW * (nch + 1)],
                                             AF.Exp,
                                             accum_out=sp[:, nch:nch + 1])
                    nc.vector.reduce_sum(s_all[:, r:r + 1], sp[:],
                                         axis=mybir.AxisListType.X)
                    nc.sync.dma_start(out_d.ap()[128 * r:128 * (r + 1), :], lg[:])
                nc.scalar.dma_start(s_out_d.ap()[:, :], s_all[:])

    nc.compile()
    return nc


_CACHE = {}


def _get(B, H, V, DEPTH):
    key = (B, H, V, DEPTH)
    if key not in _CACHE:
        _CACHE[key] = _build(B, H, V, DEPTH)
    return _CACHE[key]


def _pack_inputs(B, H, V, DEPTH, encoding, Whh_l, bih_l, bhh_l, Whh_r, bih_r,
                 bhh_r, W_out, b_out):
    """Host-side shard + transpose + cast. Returns in_maps for the 8 cores."""
    KT = H // 128
    Bl = B // N_CORES
    Vs = V // N_CORES

    woT = np.ascontiguousarray(W_out.T).astype(BFNP)          # [H, V]
    encT = np.ascontiguousarray(encoding.T).astype(BFNP)      # [H, B]

    shared = {}
    for s, Whh, bih, bhh in (("l", Whh_l, bih_l, bhh_l), ("r", Whh_r, bih_r, bhh_r)):
        shared[f"wt_{s}"] = np.ascontiguousarray(Whh.T).astype(BFNP)  # [H, 3H]
        # bias row folded into the matmul: sigmoid gates get bih+bhh,
        # candidate gate gets bhh only (bih_n is added after the r-multiply)
        wb = np.concatenate([(bih + bhh)[:2 * H], bhh[2 * H:]])
        shared[f"wb_{s}"] = wb.reshape(1, 3 * H).astype(BFNP)
        # bih_n broadcast-materialized [128, KT, 128]
        b2 = bih[2 * H:].reshape(KT, 128).T.astype(BFNP)      # [128, KT]
        shared[f"bih2_{s}"] = np.ascontiguousarray(
            np.broadcast_to(b2[:, :, None], (128, KT, 128)))

    in_maps = []
    for c in range(N_CORES):
        m = dict(shared)
        m["encT"] = np.ascontiguousarray(encT[:, c * Bl:(c + 1) * Bl])
        m["woT"] = np.ascontiguousarray(woT[:, c * Vs:(c + 1) * Vs])
        m["bo"] = np.ascontiguousarray(np.broadcast_to(
            b_out[c * Vs:(c + 1) * Vs].astype(np.float32), (128, Vs)))
        in_maps.append(m)
    return in_maps


def _run(B, H, V, DEPTH, inputs, trace=False, nc=None):
    if nc is None:
        nc = _get(B, H, V, DEPTH)
    in_maps = _pack_inputs(B, H, V, DEPTH, **inputs)
    res = bass_utils.run_bass_kernel_spmd(
        nc, in_maps, core_ids=list(range(N_CORES)), trace=trace)

    L = 1 << DEPTH
    Bl = B // N_CORES
    Vs = V // N_CORES
    # leaf column order per core: col = jj*Bl + e with jj = bitrev(true leaf)
    rev = np.array([int(format(t, f"0{DEPTH}b")[::-1], 2) for t in range(L)])
    # log-softmax denominator: sum the per-shard exp-sums across cores
    s_tot = np.zeros((B * L,), np.float64)
    for c in range(N_CORES):
        s = res.results[c]["s_out"]                  # [128, RT]
        s_tot += s.T.reshape(-1).astype(np.float64)  # row = rt*128 + p
    lse = np.log(s_tot).astype(np.float32)           # [B*L] in device row order
    lse = lse.reshape(N_CORES, L, Bl).transpose(0, 2, 1).reshape(B, L)[:, rev]
    full = np.empty((B, L, V), np.float32)
    for c in range(N_CORES):
        o = res.results[c]["out"]                    # [B*L, Vs]
        o = o.reshape(N_CORES, L, Bl, Vs)            # [src_core, jj, e, v]
        o = o.transpose(0, 2, 1, 3).reshape(B, L, Vs)
        full[:, :, c * Vs:(c + 1) * Vs] = o[:, rev, :] - lse[:, :, None]
    return full, res


def kernel(**inputs):
    enc = np.asarray(inputs["encoding"], np.float32)
    B, H = enc.shape
    V = np.asarray(inputs["W_out"]).shape[0]
    DEPTH = int(inputs["depth"])
    args = {k: np.asarray(v, np.float32) for k, v in inputs.items() if k != "depth"}
    full, _ = _run(B, H, V, DEPTH, args)
    return full


# revision 24
# speedup vs baseline: 1.0624x; 1.0624x over previous
"""DecoderTreeRNN Trainium2 kernel (8 NeuronCores, single SPMD launch).

  - Tree expansion: data-parallel over batch B (8 examples/core). GRU states
    are kept transposed [H, nodes] in bf16; each level computes
    ghT = WhhT.T @ hT with the weight tiles stationary on the PE (biases
    folded in via a K=1 ones-row matmul), then gate math on ScalarE (sigmoid
    and tanh live in one ACT table set) and VectorE, operating on 4-wide
    m-tile slabs to amortize instruction overhead. Children are concatenated
    [left | right]; the resulting bit-reversed leaf order is undone on the
    host during unshard.
  - The bf16 leaf states are AllGathered (RDH) so every core holds all
    B*32 leaf rows.
  - Output projection: tensor-parallel over vocab (4000 columns/core).
    The W_outT shard stays resident in SBUF (bf16); per row tile, one
    stationary leaf tile serves all 8 vocab chunks, each accumulating over
    the 8 K-tiles in its own PSUM bank. The vocab bias is added during the
    PSUM->SBUF copy (VectorE) and exp + row-sum is fused on ScalarE via
    accum_out. Unnormalized logits stream straight out; each core also
    returns its per-row exp-sums, and -log(sum over cores) is folded into
    the host-side unshard pass (the only cross-core softmax traffic).
  DMA discipline: the two HWDGE rings (SP + ACT) are used as ordered FIFOs -
  latency-critical small inputs and the right-side tree weights go on the
  ACT ring, the left-side tree weights followed by the projection weights
  on the SP ring, so compute starts as soon as the first tiles land.
"""

import sys

for _p in ("/opt/trn_rl_repo",):
    if _p not in sys.path:
        sys.path.append(_p)

import numpy as np
import ml_dtypes

import concourse.bass as bass
from concourse import bacc, tile, mybir
from concourse import bass_utils
from contextlib import ExitStack

BF16 = mybir.dt.bfloat16
F32 = mybir.dt.float32
AF = mybir.ActivationFunctionType
ALU = mybir.AluOpType
BFNP = ml_dtypes.bfloat16
FP8 = mybir.dt.float8e4
FP8_AG = False  # fp8 all-gather: no faster (latency-bound), costs precision

N_CORES = 8
CW = 500  # vocab chunk width (<=512 fp32 psum bank)


def _build(B, H, V, DEPTH):
    KT = H // 128            # contraction tiles
    MT = 3 * KT              # output m-tiles per GRU side
    Bl = B // N_CORES        # examples per core
    L = 1 << DEPTH           # leaves per example
    NLOC = Bl * L            # local leaf count
    ROWS = B * L             # total leaf rows
    RT = ROWS // 128         # row tiles
    Vs = V // N_CORES        # vocab shard
    NCH = Vs // CW           # chunks per shard
    SG = min(4, KT)          # m-tiles per gate slab
    NSL = KT // SG           # slabs per gate
    assert B % N_CORES == 0 and H % 128 == 0 and V % N_CORES == 0
    assert Vs % CW == 0 and ROWS % 128 == 0 and RT <= 512
    assert SG * 128 <= 512  # psum slab fits one bank

    nc = bacc.Bacc("TRN2", target_bir_lowering=False, debug=False,
                   num_devices=N_CORES, dynamic_dma_scratch_size=2048)

    # ---------------- DRAM I/O ----------------
    encT = nc.dram_tensor("encT", [H, Bl], BF16, kind="ExternalInput")
    wt_d, wb_d, bih2_d = {}, {}, {}
    for s in "lr":
        wt_d[s] = nc.dram_tensor(f"wt_{s}", [H, 3 * H], BF16, kind="ExternalInput")
        wb_d[s] = nc.dram_tensor(f"wb_{s}", [1, 3 * H], BF16, kind="ExternalInput")
        bih2_d[s] = nc.dram_tensor(f"bih2_{s}", [128, KT, 128], BF16,
                                   kind="ExternalInput")
    woT_d = nc.dram_tensor("woT", [H, Vs], BF16, kind="ExternalInput")
    bo_d = nc.dram_tensor("bo", [128, Vs], F32, kind="ExternalInput")
    out_d = nc.dram_tensor("out", [ROWS, Vs], F32, kind="ExternalOutput")

    AGDT = FP8 if FP8_AG else BF16
    ag_leaves = nc.dram_tensor("ag_leaves", [N_CORES * H, NLOC], AGDT,
                               kind="Internal", addr_space="Shared")
    s_out_d = nc.dram_tensor("s_out", [128, RT], F32, kind="ExternalOutput")

    rg = [list(range(N_CORES))]

    with tile.TileContext(nc) as tc, ExitStack() as ctx:
        dram = ctx.enter_context(tc.tile_pool(name="dram", bufs=1, space="DRAM"))
        wproj = ctx.enter_context(tc.tile_pool(name="wproj", bufs=1))
        cpool = ctx.enter_context(tc.tile_pool(name="const", bufs=1))

        # projection weights: resident for the whole kernel. Tiles are
        # allocated up front but their DMAs are issued after the tree weight
        # DMAs (below) so the tree isn't starved of HBM bandwidth at start.
        wo_sb = [wproj.tile([128, Vs], BF16, tag=f"wo{k}", name=f"wo{k}")
                 for k in range(KT)]
        bo_sb = cpool.tile([128, Vs], F32, tag="bo")
        ones_sb = cpool.tile([1, 128], BF16, tag="ones")
        nc.vector.memset(ones_sb[:], 1.0)

        leaves_bounce = dram.tile([H, NLOC], AGDT, tag="lvb")

        # ---------------- tree expansion ----------------
        with nc.named_scope("tree"):
            with tc.tile_pool(name="wtree", bufs=1) as wtp, \
                 tc.tile_pool(name="state", bufs=2) as stp, \
                 tc.tile_pool(name="gates", bufs=2) as gp, \
                 tc.tile_pool(name="pstree", bufs=8, space="PSUM") as pst:
                # latency-critical small inputs go on the ACT HWDGE ring so
                # they aren't stuck behind the big weight loads (SP ring FIFO)
                cur = stp.tile([128, KT, Bl], BF16, tag="st")
                nc.scalar.dma_start(cur[:], encT.ap().rearrange("(k p) b -> p k b", k=KT))
                wt_sb, wb_sb, bih2_sb = {}, {}, {}
                for s in "lr":
                    wb_sb[s] = wtp.tile([1, 3 * H], BF16, tag=f"wb{s}", name=f"wb_sb_{s}")
                    nc.scalar.dma_start(wb_sb[s][:], wb_d[s].ap())
                    bih2_sb[s] = wtp.tile([128, KT, 128], BF16, tag=f"bi{s}", name=f"bih2_sb_{s}")
                    nc.scalar.dma_start(bih2_sb[s][:], bih2_d[s].ap())
                # weight loads in consumption order: side l, side r, then the
                # projection weights behind them (all FIFO on the SP ring)
                for s in "lr":
                    eng = nc.sync if s == "l" else nc.scalar
                    wt_sb[s] = []
                    for k in range(KT):
                        t = wtp.tile([128, 3 * H], BF16, tag=f"wt{s}{k}")
                        eng.dma_start(t[:], wt_d[s].ap()[128 * k:128 * (k + 1), :])
                        wt_sb[s].append(t)
                for k in range(KT):
                    nc.sync.dma_start(wo_sb[k][:], woT_d.ap()[128 * k:128 * (k + 1), :])
                nc.sync.dma_start(bo_sb[:], bo_d.ap())

                n = Bl
                for lvl in range(DEPTH):
                    nxt = stp.tile([128, KT, 2 * n], BF16, tag="st")
                    for si, s in enumerate("lr"):
                        for sl in range(NSL):
                            ko0 = sl * SG
                            ps = {}
                            for gi, mb in (("r", ko0), ("z", KT + ko0), ("g", 2 * KT + ko0)):
                                p = pst.tile([128, SG, n], F32, tag="ps")
                                for mj in range(SG):
                                    m = mb + mj
                                    for k in range(KT):
                                        nc.tensor.matmul(
                                            p[:, mj, :],
                                            wt_sb[s][k][:, 128 * m:128 * (m + 1)],
                                            cur[:, k, :n],
                                            start=(k == 0), stop=False)
                                    nc.tensor.matmul(
                                        p[:, mj, :],
                                        wb_sb[s][:, 128 * m:128 * (m + 1)],
                                        ones_sb[:, :n],
                                        start=False, stop=True)
                                ps[gi] = p
                            r_t = gp.tile([128, SG, n], F32, tag="r")
                            nc.scalar.activation(r_t[:], ps["r"][:], AF.Sigmoid)
                            z_t = gp.tile([128, SG, n], F32, tag="z")
                            nc.scalar.activation(z_t[:], ps["z"][:], AF.Sigmoid)
                            t_t = gp.tile([128, SG, n], F32, tag="t")
                            nc.vector.tensor_tensor(t_t[:], ps["g"][:], r_t[:], op=ALU.mult)
                            nc.vector.tensor_tensor(
                                t_t[:], t_t[:], bih2_sb[s][:, ko0:ko0 + SG, :n], op=ALU.add)
                            n_t = gp.tile([128, SG, n], F32, tag="n")
                            nc.scalar.activation(n_t[:], t_t[:], AF.Tanh)
                            u_t = gp.tile([128, SG, n], F32, tag="u")
                            nc.vector.scalar_tensor_tensor(
                                u_t[:], n_t[:], -1.0, cur[:, ko0:ko0 + SG, :n],
                                op0=ALU.mult, op1=ALU.add)  # u = h - n
                            nc.vector.tensor_tensor(u_t[:], u_t[:], z_t[:], op=ALU.mult)
                            nc.vector.tensor_tensor(
                                nxt[:, ko0:ko0 + SG, si * n:si * n + n],
                                u_t[:], n_t[:], op=ALU.add)
                    cur = nxt
                    n *= 2

                if FP8_AG:
                    lv8 = stp.tile([128, KT, NLOC], FP8, tag="lv8", name="lv8", bufs=1)
                    nc.vector.tensor_copy(lv8[:], cur[:])
                    src_t = lv8
                else:
                    src_t = cur
                for k in range(KT):
                    eng = nc.sync if k % 2 == 0 else nc.scalar
                    eng.dma_start(leaves_bounce[128 * k:128 * (k + 1), :],
                                  src_t[:, k, :])

        # ---------------- leaves all-gather ----------------
        with nc.named_scope("ag_leaves"):
            nc.gpsimd.collective_compute(
                "AllGather", ALU.bypass, replica_groups=rg,
                ins=[leaves_bounce.opt()], outs=[ag_leaves.ap()])

        # ---------------- projection + log-softmax ----------------
        with nc.named_scope("proj"):
            with tc.tile_pool(name="leaves", bufs=1) as lvp, \
                 tc.tile_pool(name="logits", bufs=3) as lgp, \
                 tc.tile_pool(name="scr", bufs=4) as scp, \
                 tc.tile_pool(name="stats", bufs=2) as sp2, \
                 tc.tile_pool(name="psproj", bufs=8, space="PSUM") as psp:
                lv_sb = []
                ag_view = ag_leaves.ap().rearrange("(c h) j -> h c j", c=N_CORES)
                for k in range(KT):
                    t = lvp.tile([128, N_CORES, NLOC], AGDT, tag=f"lv{k}")
                    eng = nc.sync if k % 2 == 0 else nc.scalar
                    eng.dma_start(t[:], ag_view[128 * k:128 * (k + 1)])
                    lv_sb.append(t)

                # unnormalized logits stream out as soon as each row tile is
                # done; the per-shard softmax denominators are returned as a
                # tiny second output and log(sum) is folded into the host-side
                # unshard pass.
                s_all = sp2.tile([128, RT], F32, tag="sall", name="s_all")
                for r in range(RT):
                    lg = lgp.tile([128, Vs], F32, tag="lg", name=f"lg{r}")
                    sp = sp2.tile([128, NCH], F32, tag="spart", name=f"sp{r}")
                    # k-outer so one stationary (leaves) tile serves all NCH
                    # chunks; each chunk accumulates in its own PSUM bank
                    pps = [psp.tile([128, CW], F32, tag="pp", name=f"pp{r}_{nch}")
                           for nch in range(NCH)]
                    for k in range(KT):
                        lhsT = lv_sb[k][:].rearrange("p c j -> p (c j)")[
                            :, 128 * r:128 * (r + 1)]
                        for nch in range(NCH):
                            nc.tensor.matmul(
                                pps[nch][:], lhsT,
                                wo_sb[k][:, CW * nch:CW * (nch + 1)],
                                start=(k == 0), stop=(k == KT - 1))
                    for nch in range(NCH):
                        # bias add fused into the PSUM->SBUF copy
                        nc.vector.tensor_tensor(
                            lg[:, CW * nch:CW * (nch + 1)], pps[nch][:],
                            bo_sb[:, CW * nch:CW * (nch + 1)],
                            op=ALU.add)
                        ex = scp.tile([128, CW], BF16, tag="exp",
                                      name=f"ex{r}_{nch}")
                        nc.scalar.activation(ex[:],
                                             lg[:, CW * nch:CW * (nch + 1)],
                                             AF.Exp,
                                             accum_out=sp[:, nch:nch + 1])
                    nc.vector.reduce_sum(s_all[:, r:r + 1], sp[:],
                                         axis=mybir.AxisListType.X)
                    nc.sync.dma_start(out_d.ap()[128 * r:128 * (r + 1), :], lg[:])
                nc.scalar.dma_start(s_out_d.ap()[:, :], s_all[:])

    nc.compile()
    return nc


_CACHE = {}


def _get(B, H, V, DEPTH):
    key = (B, H, V, DEPTH)
    if key not in _CACHE:
        _CACHE[key] = _build(B, H, V, DEPTH)
    return _CACHE[key]


def _pack_inputs(B, H, V, DEPTH, encoding, Whh_l, bih_l, bhh_l, Whh_r, bih_r,
                 bhh_r, W_out, b_out):
    """Host-side shard + transpose + cast. Returns in_maps for the 8 cores."""
    KT = H // 128
    Bl = B // N_CORES
    Vs = V // N_CORES

    woT = np.ascontiguousarray(W_out.T).astype(BFNP)          # [H, V]
    encT = np.ascontiguousarray(encoding.T).astype(BFNP)      # [H, B]

    shared = {}
    for s, Whh, bih, bhh in (("l", Whh_l, bih_l, bhh_l), ("r", Whh_r, bih_r, bhh_r)):
        shared[f"wt_{s}"] = np.ascontiguousarray(Whh.T).astype(BFNP)  # [H, 3H]
        # bias row folded into the matmul: sigmoid gates get bih+bhh,
        # candidate gate gets bhh only (bih_n is added after the r-multiply)
        wb = np.concatenate([(bih + bhh)[:2 * H], bhh[2 * H:]])
        shared[f"wb_{s}"] = wb.reshape(1, 3 * H).astype(BFNP)
        # bih_n broadcast-materialized [128, KT, 128]
        b2 = bih[2 * H:].reshape(KT, 128).T.astype(BFNP)      # [128, KT]
        shared[f"bih2_{s}"] = np.ascontiguousarray(
            np.broadcast_to(b2[:, :, None], (128, KT, 128)))

    in_maps = []
    for c in range(N_CORES):
        m = dict(shared)
        m["encT"] = np.ascontiguousarray(encT[:, c * Bl:(c + 1) * Bl])
        m["woT"] = np.ascontiguousarray(woT[:, c * Vs:(c + 1) * Vs])
        m["bo"] = np.ascontiguousarray(np.broadcast_to(
            b_out[c * Vs:(c + 1) * Vs].astype(np.float32), (128, Vs)))
        in_maps.append(m)
    return in_maps


def _run(B, H, V, DEPTH, inputs, trace=False, nc=None):
    if nc is None:
        nc = _get(B, H, V, DEPTH)
    in_maps = _pack_inputs(B, H, V, DEPTH, **inputs)
    res = bass_utils.run_bass_kernel_spmd(
        nc, in_maps, core_ids=list(range(N_CORES)), trace=trace)

    L = 1 << DEPTH
    Bl = B // N_CORES
    Vs = V // N_CORES
    # leaf column order per core: col = jj*Bl + e with jj = bitrev(true leaf)
    rev = np.array([int(format(t, f"0{DEPTH}b")[::-1], 2) for t in range(L)])
    # log-softmax denominator: sum the per-shard exp-sums across cores
    s_tot = np.zeros((B * L,), np.float64)
    for c in range(N_CORES):
        s = res.results[c]["s_out"]                  # [128, RT]
        s_tot += s.T.reshape(-1).astype(np.float64)  # row = rt*128 + p
    lse = np.log(s_tot).astype(np.float32)           # [B*L] in device row order
    lse = lse.reshape(N_CORES, L, Bl).transpose(0, 2, 1).reshape(B, L)[:, rev]
    full = np.empty((B, L, V), np.float32)
    for c in range(N_CORES):
        o = res.results[c]["out"]                    # [B*L, Vs]
        o = o.reshape(N_CORES, L, Bl, Vs)            # [src_core, jj, e, v]
        o = o.transpose(0, 2, 1, 3).reshape(B, L, Vs)
        full[:, :, c * Vs:(c + 1) * Vs] = o[:, rev, :] - lse[:, :, None]
    return full, res


def kernel(**inputs):
    enc = np.asarray(inputs["encoding"], np.float32)
    B, H = enc.shape
    V = np.asarray(inputs["W_out"]).shape[0]
    DEPTH = int(inputs["depth"])
    args = {k: np.asarray(v, np.float32) for k, v in inputs.items() if k != "depth"}
    full, _ = _run(B, H, V, DEPTH, args)
    return full


# revision 25
# speedup vs baseline: 1.0977x; 1.0333x over previous
"""DecoderTreeRNN Trainium2 kernel (8 NeuronCores, single SPMD launch).

  - Tree expansion: data-parallel over batch B (8 examples/core). GRU states
    are kept transposed [H, nodes] in bf16; each level computes
    ghT = WhhT.T @ hT with the weight tiles stationary on the PE (biases
    folded in via a K=1 ones-row matmul), then gate math on ScalarE (sigmoid
    and tanh live in one ACT table set) and VectorE, operating on 4-wide
    m-tile slabs to amortize instruction overhead. Children are concatenated
    [left | right]; the resulting bit-reversed leaf order is undone on the
    host during unshard.
  - The bf16 leaf states are AllGathered (RDH) so every core holds all
    B*32 leaf rows.
  - Output projection: tensor-parallel over vocab (4000 columns/core).
    The W_outT shard stays resident in SBUF (bf16); per row tile, one
    stationary leaf tile serves all 8 vocab chunks, each accumulating over
    the 8 K-tiles in its own PSUM bank. The vocab bias is added during the
    PSUM->SBUF copy (VectorE) and exp + row-sum is fused on ScalarE via
    accum_out. Unnormalized logits stream straight out; each core also
    returns its per-row exp-sums, and -log(sum over cores) is folded into
    the host-side unshard pass (the only cross-core softmax traffic).
  DMA discipline: the two HWDGE rings (SP + ACT) are used as ordered FIFOs -
  latency-critical small inputs and the right-side tree weights go on the
  ACT ring, the left-side tree weights followed by the projection weights
  on the SP ring, so compute starts as soon as the first tiles land.
"""

import sys

for _p in ("/opt/trn_rl_repo",):
    if _p not in sys.path:
        sys.path.append(_p)

import numpy as np
import ml_dtypes

import concourse.bass as bass
from concourse import bacc, tile, mybir
from concourse import bass_utils
from contextlib import ExitStack

BF16 = mybir.dt.bfloat16
F32 = mybir.dt.float32
AF = mybir.ActivationFunctionType
ALU = mybir.AluOpType
BFNP = ml_dtypes.bfloat16
FP8 = mybir.dt.float8e4
FP8_AG = False  # fp8 all-gather: no faster (latency-bound), costs precision

N_CORES = 8
CW = 500  # vocab chunk width (<=512 fp32 psum bank)


def _build(B, H, V, DEPTH):
    KT = H // 128            # contraction tiles
    MT = 3 * KT              # output m-tiles per GRU side
    Bl = B // N_CORES        # examples per core
    L = 1 << DEPTH           # leaves per example
    NLOC = Bl * L            # local leaf count
    ROWS = B * L             # total leaf rows
    RT = ROWS // 128         # row tiles
    Vs = V // N_CORES        # vocab shard
    NCH = Vs // CW           # chunks per shard
    SG = min(4, KT)          # m-tiles per gate slab
    NSL = KT // SG           # slabs per gate
    assert B % N_CORES == 0 and H % 128 == 0 and V % N_CORES == 0
    assert Vs % CW == 0 and ROWS % 128 == 0 and RT <= 512
    assert SG * 128 <= 512  # psum slab fits one bank

    nc = bacc.Bacc("TRN2", target_bir_lowering=False, debug=False,
                   num_devices=N_CORES, dynamic_dma_scratch_size=2048)

    # ---------------- DRAM I/O ----------------
    encT = nc.dram_tensor("encT", [H, Bl], BF16, kind="ExternalInput")
    wt_d, wb_d, bih2_d = {}, {}, {}
    for s in "lr":
        wt_d[s] = nc.dram_tensor(f"wt_{s}", [H, 3 * H], FP8, kind="ExternalInput")
        wb_d[s] = nc.dram_tensor(f"wb_{s}", [1, 3 * H], BF16, kind="ExternalInput")
        bih2_d[s] = nc.dram_tensor(f"bih2_{s}", [128, KT, 128], BF16,
                                   kind="ExternalInput")
    woT_d = nc.dram_tensor("woT", [H, Vs], BF16, kind="ExternalInput")
    bo_d = nc.dram_tensor("bo", [128, Vs], F32, kind="ExternalInput")
    out_d = nc.dram_tensor("out", [ROWS, Vs], F32, kind="ExternalOutput")

    AGDT = FP8 if FP8_AG else BF16
    ag_leaves = nc.dram_tensor("ag_leaves", [N_CORES * H, NLOC], AGDT,
                               kind="Internal", addr_space="Shared")
    s_out_d = nc.dram_tensor("s_out", [128, RT], F32, kind="ExternalOutput")

    rg = [list(range(N_CORES))]

    with tile.TileContext(nc) as tc, ExitStack() as ctx:
        dram = ctx.enter_context(tc.tile_pool(name="dram", bufs=1, space="DRAM"))
        wproj = ctx.enter_context(tc.tile_pool(name="wproj", bufs=1))
        cpool = ctx.enter_context(tc.tile_pool(name="const", bufs=1))

        # projection weights: resident for the whole kernel. Tiles are
        # allocated up front but their DMAs are issued after the tree weight
        # DMAs (below) so the tree isn't starved of HBM bandwidth at start.
        wo_sb = [wproj.tile([128, Vs], BF16, tag=f"wo{k}", name=f"wo{k}")
                 for k in range(KT)]
        bo_sb = cpool.tile([128, Vs], F32, tag="bo")
        ones_sb = cpool.tile([1, 128], BF16, tag="ones")
        nc.vector.memset(ones_sb[:], 1.0)

        leaves_bounce = dram.tile([H, NLOC], AGDT, tag="lvb")

        # ---------------- tree expansion ----------------
        with nc.named_scope("tree"):
            with tc.tile_pool(name="wtree", bufs=1) as wtp, \
                 tc.tile_pool(name="state", bufs=2) as stp, \
                 tc.tile_pool(name="gates", bufs=2) as gp, \
                 tc.tile_pool(name="pstree", bufs=8, space="PSUM") as pst:
                # latency-critical small inputs go on the ACT HWDGE ring so
                # they aren't stuck behind the big weight loads (SP ring FIFO)
                cur = stp.tile([128, KT, Bl], BF16, tag="st")
                nc.scalar.dma_start(cur[:], encT.ap().rearrange("(k p) b -> p k b", k=KT))
                wt_sb, wb_sb, bih2_sb = {}, {}, {}
                for s in "lr":
                    wb_sb[s] = wtp.tile([1, 3 * H], BF16, tag=f"wb{s}", name=f"wb_sb_{s}")
                    nc.scalar.dma_start(wb_sb[s][:], wb_d[s].ap())
                    bih2_sb[s] = wtp.tile([128, KT, 128], BF16, tag=f"bi{s}", name=f"bih2_sb_{s}")
                    nc.scalar.dma_start(bih2_sb[s][:], bih2_d[s].ap())
                # weight loads in consumption order: side l, side r, then the
                # projection weights behind them (all FIFO on the SP ring)
                for s in "lr":
                    eng = nc.sync if s == "l" else nc.scalar
                    wt_sb[s] = []
                    for k in range(KT):
                        t = wtp.tile([128, 3 * H], FP8, tag=f"wt{s}{k}")
                        eng.dma_start(t[:], wt_d[s].ap()[128 * k:128 * (k + 1), :])
                        wt_sb[s].append(t)
                for k in range(KT):
                    nc.sync.dma_start(wo_sb[k][:], woT_d.ap()[128 * k:128 * (k + 1), :])
                nc.sync.dma_start(bo_sb[:], bo_d.ap())

                n = Bl
                for lvl in range(DEPTH):
                    nxt = stp.tile([128, KT, 2 * n], BF16, tag="st")
                    for si, s in enumerate("lr"):
                        for sl in range(NSL):
                            ko0 = sl * SG
                            ps = {}
                            for gi, mb in (("r", ko0), ("z", KT + ko0), ("g", 2 * KT + ko0)):
                                p = pst.tile([128, SG, n], F32, tag="ps")
                                for mj in range(SG):
                                    m = mb + mj
                                    for k in range(KT):
                                        nc.tensor.matmul(
                                            p[:, mj, :],
                                            wt_sb[s][k][:, 128 * m:128 * (m + 1)],
                                            cur[:, k, :n],
                                            start=(k == 0), stop=False)
                                    nc.tensor.matmul(
                                        p[:, mj, :],
                                        wb_sb[s][:, 128 * m:128 * (m + 1)],
                                        ones_sb[:, :n],
                                        start=False, stop=True)
                                ps[gi] = p
                            r_t = gp.tile([128, SG, n], F32, tag="r")
                            nc.scalar.activation(r_t[:], ps["r"][:], AF.Sigmoid)
                            z_t = gp.tile([128, SG, n], F32, tag="z")
                            nc.scalar.activation(z_t[:], ps["z"][:], AF.Sigmoid)
                            t_t = gp.tile([128, SG, n], F32, tag="t")
                            nc.vector.tensor_tensor(t_t[:], ps["g"][:], r_t[:], op=ALU.mult)
                            nc.vector.tensor_tensor(
                                t_t[:], t_t[:], bih2_sb[s][:, ko0:ko0 + SG, :n], op=ALU.add)
                            n_t = gp.tile([128, SG, n], F32, tag="n")
                            nc.scalar.activation(n_t[:], t_t[:], AF.Tanh)
                            u_t = gp.tile([128, SG, n], F32, tag="u")
                            nc.vector.scalar_tensor_tensor(
                                u_t[:], n_t[:], -1.0, cur[:, ko0:ko0 + SG, :n],
                                op0=ALU.mult, op1=ALU.add)  # u = h - n
                            nc.vector.tensor_tensor(u_t[:], u_t[:], z_t[:], op=ALU.mult)
                            nc.vector.tensor_tensor(
                                nxt[:, ko0:ko0 + SG, si * n:si * n + n],
                                u_t[:], n_t[:], op=ALU.add)
                    cur = nxt
                    n *= 2

                if FP8_AG:
                    lv8 = stp.tile([128, KT, NLOC], FP8, tag="lv8", name="lv8", bufs=1)
                    nc.vector.tensor_copy(lv8[:], cur[:])
                    src_t = lv8
                else:
                    src_t = cur
                for k in range(KT):
                    eng = nc.sync if k % 2 == 0 else nc.scalar
                    eng.dma_start(leaves_bounce[128 * k:128 * (k + 1), :],
                                  src_t[:, k, :])

        # ---------------- leaves all-gather ----------------
        with nc.named_scope("ag_leaves"):
            nc.gpsimd.collective_compute(
                "AllGather", ALU.bypass, replica_groups=rg,
                ins=[leaves_bounce.opt()], outs=[ag_leaves.ap()])

        # ---------------- projection + log-softmax ----------------
        with nc.named_scope("proj"):
            with tc.tile_pool(name="leaves", bufs=1) as lvp, \
                 tc.tile_pool(name="logits", bufs=3) as lgp, \
                 tc.tile_pool(name="scr", bufs=4) as scp, \
                 tc.tile_pool(name="stats", bufs=2) as sp2, \
                 tc.tile_pool(name="psproj", bufs=8, space="PSUM") as psp:
                lv_sb = []
                ag_view = ag_leaves.ap().rearrange("(c h) j -> h c j", c=N_CORES)
                for k in range(KT):
                    t = lvp.tile([128, N_CORES, NLOC], AGDT, tag=f"lv{k}")
                    eng = nc.sync if k % 2 == 0 else nc.scalar
                    eng.dma_start(t[:], ag_view[128 * k:128 * (k + 1)])
                    lv_sb.append(t)

                # unnormalized logits stream out as soon as each row tile is
                # done; the per-shard softmax denominators are returned as a
                # tiny second output and log(sum) is folded into the host-side
                # unshard pass.
                s_all = sp2.tile([128, RT], F32, tag="sall", name="s_all")
                for r in range(RT):
                    lg = lgp.tile([128, Vs], F32, tag="lg", name=f"lg{r}")
                    sp = sp2.tile([128, NCH], F32, tag="spart", name=f"sp{r}")
                    # k-outer so one stationary (leaves) tile serves all NCH
                    # chunks; each chunk accumulates in its own PSUM bank
                    pps = [psp.tile([128, CW], F32, tag="pp", name=f"pp{r}_{nch}")
                           for nch in range(NCH)]
                    for k in range(KT):
                        lhsT = lv_sb[k][:].rearrange("p c j -> p (c j)")[
                            :, 128 * r:128 * (r + 1)]
                        for nch in range(NCH):
                            nc.tensor.matmul(
                                pps[nch][:], lhsT,
                                wo_sb[k][:, CW * nch:CW * (nch + 1)],
                                start=(k == 0), stop=(k == KT - 1))
                    for nch in range(NCH):
                        # bias add fused into the PSUM->SBUF copy
                        nc.vector.tensor_tensor(
                            lg[:, CW * nch:CW * (nch + 1)], pps[nch][:],
                            bo_sb[:, CW * nch:CW * (nch + 1)],
                            op=ALU.add)
                        ex = scp.tile([128, CW], BF16, tag="exp",
                                      name=f"ex{r}_{nch}")
                        nc.scalar.activation(ex[:],
                                             lg[:, CW * nch:CW * (nch + 1)],
                                             AF.Exp,
                                             accum_out=sp[:, nch:nch + 1])
                    nc.vector.reduce_sum(s_all[:, r:r + 1], sp[:],
                                         axis=mybir.AxisListType.X)
                    nc.sync.dma_start(out_d.ap()[128 * r:128 * (r + 1), :], lg[:])
                nc.scalar.dma_start(s_out_d.ap()[:, :], s_all[:])

    nc.compile()
    return nc


_CACHE = {}


def _get(B, H, V, DEPTH):
    key = (B, H, V, DEPTH)
    if key not in _CACHE:
        _CACHE[key] = _build(B, H, V, DEPTH)
    return _CACHE[key]


def _pack_inputs(B, H, V, DEPTH, encoding, Whh_l, bih_l, bhh_l, Whh_r, bih_r,
                 bhh_r, W_out, b_out):
    """Host-side shard + transpose + cast. Returns in_maps for the 8 cores."""
    KT = H // 128
    Bl = B // N_CORES
    Vs = V // N_CORES

    woT = np.ascontiguousarray(W_out.T).astype(BFNP)          # [H, V]
    encT = np.ascontiguousarray(encoding.T).astype(BFNP)      # [H, B]

    shared = {}
    for s, Whh, bih, bhh in (("l", Whh_l, bih_l, bhh_l), ("r", Whh_r, bih_r, bhh_r)):
        shared[f"wt_{s}"] = np.ascontiguousarray(Whh.T).astype(
            mybir.dt.np(FP8))  # [H, 3H] fp8: weight-load bound, not precision bound
        # bias row folded into the matmul: sigmoid gates get bih+bhh,
        # candidate gate gets bhh only (bih_n is added after the r-multiply)
        wb = np.concatenate([(bih + bhh)[:2 * H], bhh[2 * H:]])
        shared[f"wb_{s}"] = wb.reshape(1, 3 * H).astype(BFNP)
        # bih_n broadcast-materialized [128, KT, 128]
        b2 = bih[2 * H:].reshape(KT, 128).T.astype(BFNP)      # [128, KT]
        shared[f"bih2_{s}"] = np.ascontiguousarray(
            np.broadcast_to(b2[:, :, None], (128, KT, 128)))

    in_maps = []
    for c in range(N_CORES):
        m = dict(shared)
        m["encT"] = np.ascontiguousarray(encT[:, c * Bl:(c + 1) * Bl])
        m["woT"] = np.ascontiguousarray(woT[:, c * Vs:(c + 1) * Vs])
        m["bo"] = np.ascontiguousarray(np.broadcast_to(
            b_out[c * Vs:(c + 1) * Vs].astype(np.float32), (128, Vs)))
        in_maps.append(m)
    return in_maps


def _run(B, H, V, DEPTH, inputs, trace=False, nc=None):
    if nc is None:
        nc = _get(B, H, V, DEPTH)
    in_maps = _pack_inputs(B, H, V, DEPTH, **inputs)
    res = bass_utils.run_bass_kernel_spmd(
        nc, in_maps, core_ids=list(range(N_CORES)), trace=trace)

    L = 1 << DEPTH
    Bl = B // N_CORES
    Vs = V // N_CORES
    # leaf column order per core: col = jj*Bl + e with jj = bitrev(true leaf)
    rev = np.array([int(format(t, f"0{DEPTH}b")[::-1], 2) for t in range(L)])
    # log-softmax denominator: sum the per-shard exp-sums across cores
    s_tot = np.zeros((B * L,), np.float64)
    for c in range(N_CORES):
        s = res.results[c]["s_out"]                  # [128, RT]
        s_tot += s.T.reshape(-1).astype(np.float64)  # row = rt*128 + p
    lse = np.log(s_tot).astype(np.float32)           # [B*L] in device row order
    lse = lse.reshape(N_CORES, L, Bl).transpose(0, 2, 1).reshape(B, L)[:, rev]
    full = np.empty((B, L, V), np.float32)
    for c in range(N_CORES):
        o = res.results[c]["out"]                    # [B*L, Vs]
        o = o.reshape(N_CORES, L, Bl, Vs)            # [src_core, jj, e, v]
        o = o.transpose(0, 2, 1, 3).reshape(B, L, Vs)
        full[:, :, c * Vs:(c + 1) * Vs] = o[:, rev, :] - lse[:, :, None]
    return full, res


def kernel(**inputs):
    enc = np.asarray(inputs["encoding"], np.float32)
    B, H = enc.shape
    V = np.asarray(inputs["W_out"]).shape[0]
    DEPTH = int(inputs["depth"])
    args = {k: np.asarray(v, np.float32) for k, v in inputs.items() if k != "depth"}
    full, _ = _run(B, H, V, DEPTH, args)
    return full


# revision 26
# speedup vs baseline: 1.5295x; 1.3934x over previous
"""DecoderTreeRNN Trainium2 kernel (8 NeuronCores, single SPMD launch).

  - Tree expansion: data-parallel over batch B (8 examples/core). GRU states
    are kept transposed [H, nodes] in bf16; each level computes
    ghT = WhhT.T @ hT with the weight tiles stationary on the PE (biases
    folded in via a K=1 ones-row matmul), then gate math on ScalarE (sigmoid
    and tanh live in one ACT table set) and VectorE, operating on 4-wide
    m-tile slabs to amortize instruction overhead. Children are concatenated
    [left | right]; the resulting bit-reversed leaf order is undone on the
    host during unshard.
  - The bf16 leaf states are AllGathered (RDH) so every core holds all
    B*32 leaf rows.
  - Output projection: tensor-parallel over vocab (4000 columns/core).
    The W_outT shard stays resident in SBUF (bf16); per row tile, one
    stationary leaf tile serves all 8 vocab chunks, each accumulating over
    the 8 K-tiles in its own PSUM bank. The vocab bias is added during the
    PSUM->SBUF copy (VectorE) and exp + row-sum is fused on ScalarE via
    accum_out. Unnormalized logits stream straight out; each core also
    returns its per-row exp-sums, and -log(sum over cores) is folded into
    the host-side unshard pass (the only cross-core softmax traffic).
  DMA discipline: the two HWDGE rings (SP + ACT) are used as ordered FIFOs -
  latency-critical small inputs and the right-side tree weights go on the
  ACT ring, the left-side tree weights followed by the projection weights
  on the SP ring, so compute starts as soon as the first tiles land.
"""

import sys

for _p in ("/opt/trn_rl_repo",):
    if _p not in sys.path:
        sys.path.append(_p)

import numpy as np
import ml_dtypes

import concourse.bass as bass
from concourse import bacc, tile, mybir
from concourse import bass_utils
from contextlib import ExitStack

BF16 = mybir.dt.bfloat16
F32 = mybir.dt.float32
AF = mybir.ActivationFunctionType
ALU = mybir.AluOpType
BFNP = ml_dtypes.bfloat16
FP8 = mybir.dt.float8e4
FP8_AG = True   # leaves in fp8: feeds the DoubleRow projection

N_CORES = 8
CW = 500  # vocab chunk width (<=512 fp32 psum bank)


def _build(B, H, V, DEPTH):
    KT = H // 128            # contraction tiles
    MT = 3 * KT              # output m-tiles per GRU side
    Bl = B // N_CORES        # examples per core
    L = 1 << DEPTH           # leaves per example
    NLOC = Bl * L            # local leaf count
    ROWS = B * L             # total leaf rows
    RT = ROWS // 128         # row tiles
    Vs = V // N_CORES        # vocab shard
    NCH = Vs // CW           # chunks per shard
    SG = min(4, KT)          # m-tiles per gate slab
    NSL = KT // SG           # slabs per gate
    assert B % N_CORES == 0 and H % 128 == 0 and V % N_CORES == 0
    assert Vs % CW == 0 and ROWS % 128 == 0 and RT <= 512
    assert SG * 128 <= 512  # psum slab fits one bank

    nc = bacc.Bacc("TRN2", target_bir_lowering=False, debug=False,
                   num_devices=N_CORES, dynamic_dma_scratch_size=2048)

    # ---------------- DRAM I/O ----------------
    encT = nc.dram_tensor("encT", [H, Bl], BF16, kind="ExternalInput")
    wt_d, wb_d, bih2_d = {}, {}, {}
    for s in "lr":
        wt_d[s] = nc.dram_tensor(f"wt_{s}", [H, 3 * H], FP8, kind="ExternalInput")
        wb_d[s] = nc.dram_tensor(f"wb_{s}", [1, 3 * H], BF16, kind="ExternalInput")
        bih2_d[s] = nc.dram_tensor(f"bih2_{s}", [128, KT, 128], BF16,
                                   kind="ExternalInput")
    KT2 = KT // 2            # DoubleRow k-tiles (K=256 each)
    woT_d = nc.dram_tensor("woT", [128, KT2, 2, Vs], FP8, kind="ExternalInput")
    bo_d = nc.dram_tensor("bo", [128, Vs], F32, kind="ExternalInput")
    out_d = nc.dram_tensor("out", [ROWS, Vs], F32, kind="ExternalOutput")

    AGDT = FP8 if FP8_AG else BF16
    ag_leaves = nc.dram_tensor("ag_leaves", [N_CORES * H, NLOC], AGDT,
                               kind="Internal", addr_space="Shared")
    s_out_d = nc.dram_tensor("s_out", [128, RT], F32, kind="ExternalOutput")

    rg = [list(range(N_CORES))]

    with tile.TileContext(nc) as tc, ExitStack() as ctx:
        dram = ctx.enter_context(tc.tile_pool(name="dram", bufs=1, space="DRAM"))
        wproj = ctx.enter_context(tc.tile_pool(name="wproj", bufs=1))
        cpool = ctx.enter_context(tc.tile_pool(name="const", bufs=1))

        # projection weights: resident for the whole kernel. Tiles are
        # allocated up front but their DMAs are issued after the tree weight
        # DMAs (below) so the tree isn't starved of HBM bandwidth at start.
        wo_sb = wproj.tile([128, KT2, 2, Vs], FP8, tag="wo8", name="wo8")
        bo_sb = cpool.tile([128, Vs], F32, tag="bo")
        ones_sb = cpool.tile([1, 128], BF16, tag="ones")
        nc.vector.memset(ones_sb[:], 1.0)

        leaves_bounce = dram.tile([H, NLOC], AGDT, tag="lvb")

        # ---------------- tree expansion ----------------
        with nc.named_scope("tree"):
            with tc.tile_pool(name="wtree", bufs=1) as wtp, \
                 tc.tile_pool(name="state", bufs=2) as stp, \
                 tc.tile_pool(name="gates", bufs=2) as gp, \
                 tc.tile_pool(name="pstree", bufs=8, space="PSUM") as pst:
                # latency-critical small inputs go on the ACT HWDGE ring so
                # they aren't stuck behind the big weight loads (SP ring FIFO)
                cur = stp.tile([128, KT, Bl], BF16, tag="st")
                nc.scalar.dma_start(cur[:], encT.ap().rearrange("(k p) b -> p k b", k=KT))
                wt_sb, wb_sb, bih2_sb = {}, {}, {}
                for s in "lr":
                    wb_sb[s] = wtp.tile([1, 3 * H], BF16, tag=f"wb{s}", name=f"wb_sb_{s}")
                    nc.scalar.dma_start(wb_sb[s][:], wb_d[s].ap())
                    bih2_sb[s] = wtp.tile([128, KT, 128], BF16, tag=f"bi{s}", name=f"bih2_sb_{s}")
                    nc.scalar.dma_start(bih2_sb[s][:], bih2_d[s].ap())
                # weight loads in consumption order: side l, side r, then the
                # projection weights behind them (all FIFO on the SP ring)
                for s in "lr":
                    eng = nc.sync if s == "l" else nc.scalar
                    wt_sb[s] = []
                    for k in range(KT):
                        t = wtp.tile([128, 3 * H], FP8, tag=f"wt{s}{k}")
                        eng.dma_start(t[:], wt_d[s].ap()[128 * k:128 * (k + 1), :])
                        wt_sb[s].append(t)
                nc.sync.dma_start(wo_sb[:], woT_d.ap())
                nc.sync.dma_start(bo_sb[:], bo_d.ap())

                n = Bl
                for lvl in range(DEPTH):
                    nxt = stp.tile([128, KT, 2 * n], BF16, tag="st")
                    for si, s in enumerate("lr"):
                        for sl in range(NSL):
                            ko0 = sl * SG
                            ps = {}
                            for gi, mb in (("r", ko0), ("z", KT + ko0), ("g", 2 * KT + ko0)):
                                p = pst.tile([128, SG, n], F32, tag="ps")
                                for mj in range(SG):
                                    m = mb + mj
                                    for k in range(KT):
                                        nc.tensor.matmul(
                                            p[:, mj, :],
                                            wt_sb[s][k][:, 128 * m:128 * (m + 1)],
                                            cur[:, k, :n],
                                            start=(k == 0), stop=False)
                                    nc.tensor.matmul(
                                        p[:, mj, :],
                                        wb_sb[s][:, 128 * m:128 * (m + 1)],
                                        ones_sb[:, :n],
                                        start=False, stop=True)
                                ps[gi] = p
                            r_t = gp.tile([128, SG, n], F32, tag="r")
                            nc.scalar.activation(r_t[:], ps["r"][:], AF.Sigmoid)
                            z_t = gp.tile([128, SG, n], F32, tag="z")
                            nc.scalar.activation(z_t[:], ps["z"][:], AF.Sigmoid)
                            t_t = gp.tile([128, SG, n], F32, tag="t")
                            nc.vector.tensor_tensor(t_t[:], ps["g"][:], r_t[:], op=ALU.mult)
                            nc.vector.tensor_tensor(
                                t_t[:], t_t[:], bih2_sb[s][:, ko0:ko0 + SG, :n], op=ALU.add)
                            n_t = gp.tile([128, SG, n], F32, tag="n")
                            nc.scalar.activation(n_t[:], t_t[:], AF.Tanh)
                            u_t = gp.tile([128, SG, n], F32, tag="u")
                            nc.vector.scalar_tensor_tensor(
                                u_t[:], n_t[:], -1.0, cur[:, ko0:ko0 + SG, :n],
                                op0=ALU.mult, op1=ALU.add)  # u = h - n
                            nc.vector.tensor_tensor(u_t[:], u_t[:], z_t[:], op=ALU.mult)
                            nc.vector.tensor_tensor(
                                nxt[:, ko0:ko0 + SG, si * n:si * n + n],
                                u_t[:], n_t[:], op=ALU.add)
                    cur = nxt
                    n *= 2

                if FP8_AG:
                    lv8 = stp.tile([128, KT, NLOC], FP8, tag="lv8", name="lv8", bufs=1)
                    nc.vector.tensor_copy(lv8[:], cur[:])
                    src_t = lv8
                else:
                    src_t = cur
                for k in range(KT):
                    eng = nc.sync if k % 2 == 0 else nc.scalar
                    eng.dma_start(leaves_bounce[128 * k:128 * (k + 1), :],
                                  src_t[:, k, :])

        # ---------------- leaves all-gather ----------------
        with nc.named_scope("ag_leaves"):
            nc.gpsimd.collective_compute(
                "AllGather", ALU.bypass, replica_groups=rg,
                ins=[leaves_bounce.opt()], outs=[ag_leaves.ap()])

        # ---------------- projection + log-softmax ----------------
        with nc.named_scope("proj"):
            with tc.tile_pool(name="leaves", bufs=1) as lvp, \
                 tc.tile_pool(name="logits", bufs=3) as lgp, \
                 tc.tile_pool(name="scr", bufs=4) as scp, \
                 tc.tile_pool(name="stats", bufs=2) as sp2, \
                 tc.tile_pool(name="psproj", bufs=8, space="PSUM") as psp:
                ag_view = ag_leaves.ap().rearrange("(c h) j -> h c j", c=N_CORES)
                lvbig = lvp.tile([128, KT, N_CORES * NLOC], AGDT, tag="lvbig")
                for k in range(KT):
                    eng = nc.sync if k % 2 == 0 else nc.scalar
                    eng.dma_start(
                        lvbig[:, k, :].rearrange("p (c j) -> p c j", c=N_CORES),
                        ag_view[128 * k:128 * (k + 1)])

                # unnormalized logits stream out as soon as each row tile is
                # done; the per-shard softmax denominators are returned as a
                # tiny second output and log(sum) is folded into the host-side
                # unshard pass.
                s_all = sp2.tile([128, RT], F32, tag="sall", name="s_all")
                for r in range(RT):
                    lg = lgp.tile([128, Vs], F32, tag="lg", name=f"lg{r}")
                    sp = sp2.tile([128, NCH], F32, tag="spart", name=f"sp{r}")
                    # k-outer so one stationary (leaves) tile serves all NCH
                    # chunks; each chunk accumulates in its own PSUM bank
                    pps = [psp.tile([128, CW], F32, tag="pp", name=f"pp{r}_{nch}")
                           for nch in range(NCH)]
                    for k2 in range(KT2):
                        lhsT = lvbig[:, 2 * k2:2 * k2 + 2, 128 * r:128 * (r + 1)]
                        for nch in range(NCH):
                            nc.tensor.matmul(
                                pps[nch][:], lhsT,
                                wo_sb[:, k2, :, CW * nch:CW * (nch + 1)],
                                perf_mode=mybir.MatmulPerfMode.DoubleRow,
                                start=(k2 == 0), stop=(k2 == KT2 - 1))
                    for nch in range(NCH):
                        # bias add fused into the PSUM->SBUF copy
                        nc.vector.tensor_tensor(
                            lg[:, CW * nch:CW * (nch + 1)], pps[nch][:],
                            bo_sb[:, CW * nch:CW * (nch + 1)],
                            op=ALU.add)
                        ex = scp.tile([128, CW], BF16, tag="exp",
                                      name=f"ex{r}_{nch}")
                        nc.scalar.activation(ex[:],
                                             lg[:, CW * nch:CW * (nch + 1)],
                                             AF.Exp,
                                             accum_out=sp[:, nch:nch + 1])
                    nc.vector.reduce_sum(s_all[:, r:r + 1], sp[:],
                                         axis=mybir.AxisListType.X)
                    nc.sync.dma_start(out_d.ap()[128 * r:128 * (r + 1), :], lg[:])
                nc.scalar.dma_start(s_out_d.ap()[:, :], s_all[:])

    nc.compile()
    return nc


_CACHE = {}


def _get(B, H, V, DEPTH):
    key = (B, H, V, DEPTH)
    if key not in _CACHE:
        _CACHE[key] = _build(B, H, V, DEPTH)
    return _CACHE[key]


def _pack_inputs(B, H, V, DEPTH, encoding, Whh_l, bih_l, bhh_l, Whh_r, bih_r,
                 bhh_r, W_out, b_out):
    """Host-side shard + transpose + cast. Returns in_maps for the 8 cores."""
    KT = H // 128
    Bl = B // N_CORES
    Vs = V // N_CORES

    KT2 = KT // 2
    woT = np.ascontiguousarray(W_out.T).astype(np.float32)    # [H, V]
    encT = np.ascontiguousarray(encoding.T).astype(BFNP)      # [H, B]

    shared = {}
    for s, Whh, bih, bhh in (("l", Whh_l, bih_l, bhh_l), ("r", Whh_r, bih_r, bhh_r)):
        shared[f"wt_{s}"] = np.ascontiguousarray(Whh.T).astype(
            mybir.dt.np(FP8))  # [H, 3H] fp8: weight-load bound, not precision bound
        # bias row folded into the matmul: sigmoid gates get bih+bhh,
        # candidate gate gets bhh only (bih_n is added after the r-multiply)
        wb = np.concatenate([(bih + bhh)[:2 * H], bhh[2 * H:]])
        shared[f"wb_{s}"] = wb.reshape(1, 3 * H).astype(BFNP)
        # bih_n broadcast-materialized [128, KT, 128]
        b2 = bih[2 * H:].reshape(KT, 128).T.astype(BFNP)      # [128, KT]
        shared[f"bih2_{s}"] = np.ascontiguousarray(
            np.broadcast_to(b2[:, :, None], (128, KT, 128)))

    in_maps = []
    for c in range(N_CORES):
        m = dict(shared)
        m["encT"] = np.ascontiguousarray(encT[:, c * Bl:(c + 1) * Bl])
        w = woT[:, c * Vs:(c + 1) * Vs].reshape(KT2, 2, 128, Vs)
        m["woT"] = np.ascontiguousarray(
            w.transpose(2, 0, 1, 3)).astype(mybir.dt.np(FP8))
        m["bo"] = np.ascontiguousarray(np.broadcast_to(
            b_out[c * Vs:(c + 1) * Vs].astype(np.float32), (128, Vs)))
        in_maps.append(m)
    return in_maps


def _run(B, H, V, DEPTH, inputs, trace=False, nc=None):
    if nc is None:
        nc = _get(B, H, V, DEPTH)
    in_maps = _pack_inputs(B, H, V, DEPTH, **inputs)
    res = bass_utils.run_bass_kernel_spmd(
        nc, in_maps, core_ids=list(range(N_CORES)), trace=trace)

    L = 1 << DEPTH
    Bl = B // N_CORES
    Vs = V // N_CORES
    # leaf column order per core: col = jj*Bl + e with jj = bitrev(true leaf)
    rev = np.array([int(format(t, f"0{DEPTH}b")[::-1], 2) for t in range(L)])
    # log-softmax denominator: sum the per-shard exp-sums across cores
    s_tot = np.zeros((B * L,), np.float64)
    for c in range(N_CORES):
        s = res.results[c]["s_out"]                  # [128, RT]
        s_tot += s.T.reshape(-1).astype(np.float64)  # row = rt*128 + p
    lse = np.log(s_tot).astype(np.float32)           # [B*L] in device row order
    lse = lse.reshape(N_CORES, L, Bl).transpose(0, 2, 1).reshape(B, L)[:, rev]
    full = np.empty((B, L, V), np.float32)
    for c in range(N_CORES):
        o = res.results[c]["out"]                    # [B*L, Vs]
        o = o.reshape(N_CORES, L, Bl, Vs)            # [src_core, jj, e, v]
        o = o.transpose(0, 2, 1, 3).reshape(B, L, Vs)
        full[:, :, c * Vs:(c + 1) * Vs] = o[:, rev, :] - lse[:, :, None]
    return full, res


def kernel(**inputs):
    enc = np.asarray(inputs["encoding"], np.float32)
    B, H = enc.shape
    V = np.asarray(inputs["W_out"]).shape[0]
    DEPTH = int(inputs["depth"])
    args = {k: np.asarray(v, np.float32) for k, v in inputs.items() if k != "depth"}
    full, _ = _run(B, H, V, DEPTH, args)
    return full


# revision 27
# speedup vs baseline: 1.6886x; 1.1040x over previous
"""DecoderTreeRNN Trainium2 kernel (8 NeuronCores, single SPMD launch).

  - Tree expansion: data-parallel over batch B (8 examples/core). GRU states
    are kept transposed [H, nodes] in bf16; each level computes
    ghT = WhhT.T @ hT with the weight tiles stationary on the PE (biases
    folded in via a K=1 ones-row matmul), then gate math on ScalarE (sigmoid
    and tanh live in one ACT table set) and VectorE, operating on 4-wide
    m-tile slabs to amortize instruction overhead. Children are concatenated
    [left | right]; the resulting bit-reversed leaf order is undone on the
    host during unshard.
  - The bf16 leaf states are AllGathered (RDH) so every core holds all
    B*32 leaf rows.
  - Output projection: tensor-parallel over vocab (4000 columns/core).
    The W_outT shard stays resident in SBUF (bf16); per row tile, one
    stationary leaf tile serves all 8 vocab chunks, each accumulating over
    the 8 K-tiles in its own PSUM bank. The vocab bias is added during the
    PSUM->SBUF copy (VectorE) and exp + row-sum is fused on ScalarE via
    accum_out. Unnormalized logits stream straight out; each core also
    returns its per-row exp-sums, and -log(sum over cores) is folded into
    the host-side unshard pass (the only cross-core softmax traffic).
  DMA discipline: the two HWDGE rings (SP + ACT) are used as ordered FIFOs -
  latency-critical small inputs and the right-side tree weights go on the
  ACT ring, the left-side tree weights followed by the projection weights
  on the SP ring, so compute starts as soon as the first tiles land.
"""

import sys

for _p in ("/opt/trn_rl_repo",):
    if _p not in sys.path:
        sys.path.append(_p)

import numpy as np
import ml_dtypes

import concourse.bass as bass
from concourse import bacc, tile, mybir
from concourse import bass_utils
from contextlib import ExitStack

BF16 = mybir.dt.bfloat16
F32 = mybir.dt.float32
AF = mybir.ActivationFunctionType
ALU = mybir.AluOpType
BFNP = ml_dtypes.bfloat16
FP8 = mybir.dt.float8e4
FP8_AG = True   # leaves in fp8: feeds the DoubleRow projection

N_CORES = 8
CW = 500  # vocab chunk width (<=512 fp32 psum bank)


def _build(B, H, V, DEPTH):
    KT = H // 128            # contraction tiles
    MT = 3 * KT              # output m-tiles per GRU side
    Bl = B // N_CORES        # examples per core
    L = 1 << DEPTH           # leaves per example
    NLOC = Bl * L            # local leaf count
    ROWS = B * L             # total leaf rows
    RT = ROWS // 128         # row tiles
    Vs = V // N_CORES        # vocab shard
    NCH = Vs // CW           # chunks per shard
    SG = min(4, KT)          # m-tiles per gate slab
    NSL = KT // SG           # slabs per gate
    assert B % N_CORES == 0 and H % 128 == 0 and V % N_CORES == 0
    assert Vs % CW == 0 and ROWS % 128 == 0 and RT <= 512
    assert SG * 128 <= 512  # psum slab fits one bank

    nc = bacc.Bacc("TRN2", target_bir_lowering=False, debug=False,
                   num_devices=N_CORES, dynamic_dma_scratch_size=2048)

    # ---------------- DRAM I/O ----------------
    encT = nc.dram_tensor("encT", [H, Bl], BF16, kind="ExternalInput")
    wt_d, wb_d, bih2_d = {}, {}, {}
    for s in "lr":
        wt_d[s] = nc.dram_tensor(f"wt_{s}", [H, 3 * H], FP8, kind="ExternalInput")
        wb_d[s] = nc.dram_tensor(f"wb_{s}", [128, 3 * KT], F32, kind="ExternalInput")
        bih2_d[s] = nc.dram_tensor(f"bih2_{s}", [128, KT, 128], BF16,
                                   kind="ExternalInput")
    KT2 = KT // 2            # DoubleRow k-tiles (K=256 each)
    woT_d = nc.dram_tensor("woT", [128, KT2, 2, Vs], FP8, kind="ExternalInput")
    bo_d = nc.dram_tensor("bo", [128, Vs], F32, kind="ExternalInput")
    out_d = nc.dram_tensor("out", [ROWS, Vs], F32, kind="ExternalOutput")

    AGDT = FP8 if FP8_AG else BF16
    ag_leaves = nc.dram_tensor("ag_leaves", [N_CORES * H, NLOC], AGDT,
                               kind="Internal", addr_space="Shared")
    s_out_d = nc.dram_tensor("s_out", [128, RT], F32, kind="ExternalOutput")

    rg = [list(range(N_CORES))]

    with tile.TileContext(nc) as tc, ExitStack() as ctx:
        dram = ctx.enter_context(tc.tile_pool(name="dram", bufs=1, space="DRAM"))
        wproj = ctx.enter_context(tc.tile_pool(name="wproj", bufs=1))
        cpool = ctx.enter_context(tc.tile_pool(name="const", bufs=1))

        # projection weights: resident for the whole kernel. Tiles are
        # allocated up front but their DMAs are issued after the tree weight
        # DMAs (below) so the tree isn't starved of HBM bandwidth at start.
        wo_sb = wproj.tile([128, KT2, 2, Vs], FP8, tag="wo8", name="wo8")
        bo_sb = cpool.tile([128, Vs], F32, tag="bo")
        ones_sb = cpool.tile([1, 128], BF16, tag="ones")
        nc.vector.memset(ones_sb[:], 1.0)

        leaves_bounce = dram.tile([H, NLOC], AGDT, tag="lvb")

        # ---------------- tree expansion ----------------
        with nc.named_scope("tree"):
            with tc.tile_pool(name="wtree", bufs=1) as wtp, \
                 tc.tile_pool(name="state", bufs=2) as stp, \
                 tc.tile_pool(name="gates", bufs=2) as gp, \
                 tc.tile_pool(name="pstree", bufs=8, space="PSUM") as pst:
                # latency-critical small inputs go on the ACT HWDGE ring so
                # they aren't stuck behind the big weight loads (SP ring FIFO)
                cur = stp.tile([128, KT, Bl], BF16, tag="st")
                nc.scalar.dma_start(cur[:], encT.ap().rearrange("(k p) b -> p k b", k=KT))
                wt_sb, wb_sb, bih2_sb = {}, {}, {}
                for s in "lr":
                    wb_sb[s] = wtp.tile([128, 3 * KT], F32, tag=f"wb{s}", name=f"wb_sb_{s}")
                    nc.scalar.dma_start(wb_sb[s][:], wb_d[s].ap())
                    bih2_sb[s] = wtp.tile([128, KT, 128], BF16, tag=f"bi{s}", name=f"bih2_sb_{s}")
                    nc.scalar.dma_start(bih2_sb[s][:], bih2_d[s].ap())
                # weight loads in consumption order: side l, side r, then the
                # projection weights behind them (all FIFO on the SP ring)
                for s in "lr":
                    eng = nc.sync if s == "l" else nc.scalar
                    wt_sb[s] = []
                    for k in range(KT):
                        t = wtp.tile([128, 3 * H], FP8, tag=f"wt{s}{k}")
                        eng.dma_start(t[:], wt_d[s].ap()[128 * k:128 * (k + 1), :])
                        wt_sb[s].append(t)
                nc.sync.dma_start(wo_sb[:], woT_d.ap())
                nc.sync.dma_start(bo_sb[:], bo_d.ap())

                n = Bl
                for lvl in range(DEPTH):
                    last = lvl == DEPTH - 1
                    nxt = stp.tile([128, KT, 2 * n], AGDT if last else BF16,
                                   tag="st8" if last else "st",
                                   name=f"nxt{lvl}", bufs=1 if last else None)
                    for si, s in enumerate("lr"):
                        for sl in range(NSL):
                            ko0 = sl * SG
                            ps = {}
                            for gi, mb in (("r", ko0), ("z", KT + ko0), ("g", 2 * KT + ko0)):
                                p = pst.tile([128, SG, n], F32, tag="ps")
                                for mj in range(SG):
                                    m = mb + mj
                                    for k in range(KT):
                                        nc.tensor.matmul(
                                            p[:, mj, :],
                                            wt_sb[s][k][:, 128 * m:128 * (m + 1)],
                                            cur[:, k, :n],
                                            start=(k == 0), stop=(k == KT - 1))
                                ps[gi] = p
                            # biases folded in via free-dim-broadcast adds (DVE)
                            def _bias(mb_):
                                return wb_sb[s][:, mb_:mb_ + SG].unsqueeze(2)                                    .broadcast_to((128, SG, n))
                            y_r = gp.tile([128, SG, n], F32, tag="yr")
                            nc.vector.tensor_tensor(y_r[:], ps["r"][:], _bias(ko0), op=ALU.add)
                            r_t = gp.tile([128, SG, n], F32, tag="r")
                            nc.scalar.activation(r_t[:], y_r[:], AF.Sigmoid)
                            y_z = gp.tile([128, SG, n], F32, tag="yz")
                            nc.vector.tensor_tensor(y_z[:], ps["z"][:], _bias(KT + ko0), op=ALU.add)
                            z_t = gp.tile([128, SG, n], F32, tag="z")
                            nc.scalar.activation(z_t[:], y_z[:], AF.Sigmoid)
                            y_g = gp.tile([128, SG, n], F32, tag="yg")
                            nc.vector.tensor_tensor(y_g[:], ps["g"][:], _bias(2 * KT + ko0), op=ALU.add)
                            t_t = gp.tile([128, SG, n], F32, tag="t")
                            nc.vector.tensor_tensor(t_t[:], y_g[:], r_t[:], op=ALU.mult)
                            nc.vector.tensor_tensor(
                                t_t[:], t_t[:], bih2_sb[s][:, ko0:ko0 + SG, :n], op=ALU.add)
                            n_t = gp.tile([128, SG, n], F32, tag="n")
                            nc.scalar.activation(n_t[:], t_t[:], AF.Tanh)
                            u_t = gp.tile([128, SG, n], F32, tag="u")
                            nc.vector.scalar_tensor_tensor(
                                u_t[:], n_t[:], -1.0, cur[:, ko0:ko0 + SG, :n],
                                op0=ALU.mult, op1=ALU.add)  # u = h - n
                            nc.vector.tensor_tensor(u_t[:], u_t[:], z_t[:], op=ALU.mult)
                            nc.vector.tensor_tensor(
                                nxt[:, ko0:ko0 + SG, si * n:si * n + n],
                                u_t[:], n_t[:], op=ALU.add)
                    cur = nxt
                    n *= 2

                for k in range(KT):
                    eng = nc.sync if k % 2 == 0 else nc.scalar
                    eng.dma_start(leaves_bounce[128 * k:128 * (k + 1), :],
                                  cur[:, k, :])

        # ---------------- leaves all-gather ----------------
        with nc.named_scope("ag_leaves"):
            nc.gpsimd.collective_compute(
                "AllGather", ALU.bypass, replica_groups=rg,
                ins=[leaves_bounce.opt()], outs=[ag_leaves.ap()])

        # ---------------- projection + log-softmax ----------------
        with nc.named_scope("proj"):
            with tc.tile_pool(name="leaves", bufs=1) as lvp, \
                 tc.tile_pool(name="logits", bufs=3) as lgp, \
                 tc.tile_pool(name="scr", bufs=4) as scp, \
                 tc.tile_pool(name="stats", bufs=2) as sp2, \
                 tc.tile_pool(name="psproj", bufs=8, space="PSUM") as psp:
                ag_view = ag_leaves.ap().rearrange("(c h) j -> h c j", c=N_CORES)
                lvbig = lvp.tile([128, KT, N_CORES * NLOC], AGDT, tag="lvbig")
                for k in range(KT):
                    eng = nc.sync if k % 2 == 0 else nc.scalar
                    eng.dma_start(
                        lvbig[:, k, :].rearrange("p (c j) -> p c j", c=N_CORES),
                        ag_view[128 * k:128 * (k + 1)])

                # unnormalized logits stream out as soon as each row tile is
                # done; the per-shard softmax denominators are returned as a
                # tiny second output and log(sum) is folded into the host-side
                # unshard pass.
                s_all = sp2.tile([128, RT], F32, tag="sall", name="s_all")
                for r in range(RT):
                    lg = lgp.tile([128, Vs], F32, tag="lg", name=f"lg{r}")
                    sp = sp2.tile([128, NCH], F32, tag="spart", name=f"sp{r}")
                    # k-outer so one stationary (leaves) tile serves all NCH
                    # chunks; each chunk accumulates in its own PSUM bank
                    pps = [psp.tile([128, CW], F32, tag="pp", name=f"pp{r}_{nch}")
                           for nch in range(NCH)]
                    for k2 in range(KT2):
                        lhsT = lvbig[:, 2 * k2:2 * k2 + 2, 128 * r:128 * (r + 1)]
                        for nch in range(NCH):
                            nc.tensor.matmul(
                                pps[nch][:], lhsT,
                                wo_sb[:, k2, :, CW * nch:CW * (nch + 1)],
                                perf_mode=mybir.MatmulPerfMode.DoubleRow,
                                start=(k2 == 0), stop=(k2 == KT2 - 1))
                    for nch in range(NCH):
                        # bias add fused into the PSUM->SBUF copy
                        nc.vector.tensor_tensor(
                            lg[:, CW * nch:CW * (nch + 1)], pps[nch][:],
                            bo_sb[:, CW * nch:CW * (nch + 1)],
                            op=ALU.add)
                        ex = scp.tile([128, CW], BF16, tag="exp",
                                      name=f"ex{r}_{nch}")
                        nc.scalar.activation(ex[:],
                                             lg[:, CW * nch:CW * (nch + 1)],
                                             AF.Exp,
                                             accum_out=sp[:, nch:nch + 1])
                    nc.vector.reduce_sum(s_all[:, r:r + 1], sp[:],
                                         axis=mybir.AxisListType.X)
                    nc.sync.dma_start(out_d.ap()[128 * r:128 * (r + 1), :], lg[:])
                nc.scalar.dma_start(s_out_d.ap()[:, :], s_all[:])

    nc.compile()
    return nc


_CACHE = {}


def _get(B, H, V, DEPTH):
    key = (B, H, V, DEPTH)
    if key not in _CACHE:
        _CACHE[key] = _build(B, H, V, DEPTH)
    return _CACHE[key]


def _pack_inputs(B, H, V, DEPTH, encoding, Whh_l, bih_l, bhh_l, Whh_r, bih_r,
                 bhh_r, W_out, b_out):
    """Host-side shard + transpose + cast. Returns in_maps for the 8 cores."""
    KT = H // 128
    Bl = B // N_CORES
    Vs = V // N_CORES

    KT2 = KT // 2
    woT = np.ascontiguousarray(W_out.T).astype(np.float32)    # [H, V]
    encT = np.ascontiguousarray(encoding.T).astype(BFNP)      # [H, B]

    shared = {}
    for s, Whh, bih, bhh in (("l", Whh_l, bih_l, bhh_l), ("r", Whh_r, bih_r, bhh_r)):
        shared[f"wt_{s}"] = np.ascontiguousarray(Whh.T).astype(
            mybir.dt.np(FP8))  # [H, 3H] fp8: weight-load bound, not precision bound
        # bias row folded into the matmul: sigmoid gates get bih+bhh,
        # candidate gate gets bhh only (bih_n is added after the r-multiply)
        wb = np.concatenate([(bih + bhh)[:2 * H], bhh[2 * H:]])
        shared[f"wb_{s}"] = np.ascontiguousarray(
            wb.reshape(3 * KT, 128).T.astype(np.float32))
        # bih_n broadcast-materialized [128, KT, 128]
        b2 = bih[2 * H:].reshape(KT, 128).T.astype(BFNP)      # [128, KT]
        shared[f"bih2_{s}"] = np.ascontiguousarray(
            np.broadcast_to(b2[:, :, None], (128, KT, 128)))

    in_maps = []
    for c in range(N_CORES):
        m = dict(shared)
        m["encT"] = np.ascontiguousarray(encT[:, c * Bl:(c + 1) * Bl])
        w = woT[:, c * Vs:(c + 1) * Vs].reshape(KT2, 2, 128, Vs)
        m["woT"] = np.ascontiguousarray(
            w.transpose(2, 0, 1, 3)).astype(mybir.dt.np(FP8))
        m["bo"] = np.ascontiguousarray(np.broadcast_to(
            b_out[c * Vs:(c + 1) * Vs].astype(np.float32), (128, Vs)))
        in_maps.append(m)
    return in_maps


def _run(B, H, V, DEPTH, inputs, trace=False, nc=None):
    if nc is None:
        nc = _get(B, H, V, DEPTH)
    in_maps = _pack_inputs(B, H, V, DEPTH, **inputs)
    res = bass_utils.run_bass_kernel_spmd(
        nc, in_maps, core_ids=list(range(N_CORES)), trace=trace)

    L = 1 << DEPTH
    Bl = B // N_CORES
    Vs = V // N_CORES
    # leaf column order per core: col = jj*Bl + e with jj = bitrev(true leaf)
    rev = np.array([int(format(t, f"0{DEPTH}b")[::-1], 2) for t in range(L)])
    # log-softmax denominator: sum the per-shard exp-sums across cores
    s_tot = np.zeros((B * L,), np.float64)
    for c in range(N_CORES):
        s = res.results[c]["s_out"]                  # [128, RT]
        s_tot += s.T.reshape(-1).astype(np.float64)  # row = rt*128 + p
    lse = np.log(s_tot).astype(np.float32)           # [B*L] in device row order
    lse = lse.reshape(N_CORES, L, Bl).transpose(0, 2, 1).reshape(B, L)[:, rev]
    full = np.empty((B, L, V), np.float32)
    for c in range(N_CORES):
        o = res.results[c]["out"]                    # [B*L, Vs]
        o = o.reshape(N_CORES, L, Bl, Vs)            # [src_core, jj, e, v]
        o = o.transpose(0, 2, 1, 3).reshape(B, L, Vs)
        full[:, :, c * Vs:(c + 1) * Vs] = o[:, rev, :] - lse[:, :, None]
    return full, res


def kernel(**inputs):
    enc = np.asarray(inputs["encoding"], np.float32)
    B, H = enc.shape
    V = np.asarray(inputs["W_out"]).shape[0]
    DEPTH = int(inputs["depth"])
    args = {k: np.asarray(v, np.float32) for k, v in inputs.items() if k != "depth"}
    full, _ = _run(B, H, V, DEPTH, args)
    return full


# revision 28
# speedup vs baseline: 1.7412x; 1.0312x over previous
"""DecoderTreeRNN Trainium2 kernel (8 NeuronCores, single SPMD launch).

  - Tree expansion: data-parallel over batch B (8 examples/core). GRU states
    are kept transposed [H, nodes] in bf16; each level computes
    ghT = WhhT.T @ hT with the weight tiles stationary on the PE (biases
    folded in via a K=1 ones-row matmul), then gate math on ScalarE (sigmoid
    and tanh live in one ACT table set) and VectorE, operating on 4-wide
    m-tile slabs to amortize instruction overhead. Children are concatenated
    [left | right]; the resulting bit-reversed leaf order is undone on the
    host during unshard.
  - The bf16 leaf states are AllGathered (RDH) so every core holds all
    B*32 leaf rows.
  - Output projection: tensor-parallel over vocab (4000 columns/core).
    The W_outT shard stays resident in SBUF (bf16); per row tile, one
    stationary leaf tile serves all 8 vocab chunks, each accumulating over
    the 8 K-tiles in its own PSUM bank. The vocab bias is added during the
    PSUM->SBUF copy (VectorE) and exp + row-sum is fused on ScalarE via
    accum_out. Unnormalized logits stream straight out; each core also
    returns its per-row exp-sums, and -log(sum over cores) is folded into
    the host-side unshard pass (the only cross-core softmax traffic).
  DMA discipline: the two HWDGE rings (SP + ACT) are used as ordered FIFOs -
  latency-critical small inputs and the right-side tree weights go on the
  ACT ring, the left-side tree weights followed by the projection weights
  on the SP ring, so compute starts as soon as the first tiles land.
"""

import sys

for _p in ("/opt/trn_rl_repo",):
    if _p not in sys.path:
        sys.path.append(_p)

import numpy as np
import ml_dtypes

import concourse.bass as bass
from concourse import bacc, tile, mybir
from concourse import bass_utils
from contextlib import ExitStack

BF16 = mybir.dt.bfloat16
F32 = mybir.dt.float32
AF = mybir.ActivationFunctionType
ALU = mybir.AluOpType
BFNP = ml_dtypes.bfloat16
FP8 = mybir.dt.float8e4
FP8_AG = True   # leaves in fp8: feeds the DoubleRow projection

N_CORES = 8
CW = 500  # vocab chunk width (<=512 fp32 psum bank)


def _build(B, H, V, DEPTH):
    KT = H // 128            # contraction tiles
    MT = 3 * KT              # output m-tiles per GRU side
    Bl = B // N_CORES        # examples per core
    L = 1 << DEPTH           # leaves per example
    NLOC = Bl * L            # local leaf count
    ROWS = B * L             # total leaf rows
    RT = ROWS // 128         # row tiles
    Vs = V // N_CORES        # vocab shard
    NCH = Vs // CW           # chunks per shard
    SG = min(4, KT)          # m-tiles per gate slab
    NSL = KT // SG           # slabs per gate
    assert B % N_CORES == 0 and H % 128 == 0 and V % N_CORES == 0
    assert Vs % CW == 0 and ROWS % 128 == 0 and RT <= 512
    assert SG * 128 <= 512  # psum slab fits one bank

    nc = bacc.Bacc("TRN2", target_bir_lowering=False, debug=False,
                   num_devices=N_CORES, dynamic_dma_scratch_size=2048)

    # ---------------- DRAM I/O ----------------
    encT = nc.dram_tensor("encT", [H, Bl], BF16, kind="ExternalInput")
    wt_d, wb_d, bih2_d = {}, {}, {}
    for s in "lr":
        wt_d[s] = nc.dram_tensor(f"wt_{s}", [H, 3 * H], FP8, kind="ExternalInput")
        wb_d[s] = nc.dram_tensor(f"wb_{s}", [128, 3 * KT], F32, kind="ExternalInput")
        bih2_d[s] = nc.dram_tensor(f"bih2_{s}", [128, KT], F32,
                                   kind="ExternalInput")
    KT2 = KT // 2            # DoubleRow k-tiles (K=256 each)
    woT_d = nc.dram_tensor("woT", [128, KT2, 2, Vs], FP8, kind="ExternalInput")
    bo_d = nc.dram_tensor("bo", [128, Vs], F32, kind="ExternalInput")
    out_d = nc.dram_tensor("out", [ROWS, Vs], F32, kind="ExternalOutput")

    AGDT = FP8 if FP8_AG else BF16
    ag_leaves = nc.dram_tensor("ag_leaves", [N_CORES * H, NLOC], AGDT,
                               kind="Internal", addr_space="Shared")
    s_out_d = nc.dram_tensor("s_out", [128, RT], F32, kind="ExternalOutput")

    rg = [list(range(N_CORES))]

    with tile.TileContext(nc) as tc, ExitStack() as ctx:
        dram = ctx.enter_context(tc.tile_pool(name="dram", bufs=1, space="DRAM"))
        wproj = ctx.enter_context(tc.tile_pool(name="wproj", bufs=1))
        cpool = ctx.enter_context(tc.tile_pool(name="const", bufs=1))

        # projection weights: resident for the whole kernel. Tiles are
        # allocated up front but their DMAs are issued after the tree weight
        # DMAs (below) so the tree isn't starved of HBM bandwidth at start.
        wo_sb = wproj.tile([128, KT2, 2, Vs], FP8, tag="wo8", name="wo8")
        bo_sb = cpool.tile([128, Vs], F32, tag="bo")
        ones_sb = cpool.tile([1, 128], BF16, tag="ones")
        nc.vector.memset(ones_sb[:], 1.0)

        leaves_bounce = dram.tile([H, NLOC], AGDT, tag="lvb")

        # ---------------- tree expansion ----------------
        with nc.named_scope("tree"):
            with tc.tile_pool(name="wtree", bufs=1) as wtp, \
                 tc.tile_pool(name="state", bufs=2) as stp, \
                 tc.tile_pool(name="gates", bufs=2) as gp, \
                 tc.tile_pool(name="pstree", bufs=8, space="PSUM") as pst:
                # latency-critical small inputs go on the ACT HWDGE ring so
                # they aren't stuck behind the big weight loads (SP ring FIFO)
                cur = stp.tile([128, KT, Bl], BF16, tag="st")
                nc.scalar.dma_start(cur[:], encT.ap().rearrange("(k p) b -> p k b", k=KT))
                wt_sb, wb_sb, bih2_sb = {}, {}, {}
                for s in "lr":
                    wb_sb[s] = wtp.tile([128, 3 * KT], F32, tag=f"wb{s}", name=f"wb_sb_{s}")
                    nc.scalar.dma_start(wb_sb[s][:], wb_d[s].ap())
                    bih2_sb[s] = wtp.tile([128, KT], F32, tag=f"bi{s}", name=f"bih2_sb_{s}")
                    nc.scalar.dma_start(bih2_sb[s][:], bih2_d[s].ap())
                # weight loads in consumption order: side l, side r, then the
                # projection weights behind them (all FIFO on the SP ring)
                for s in "lr":
                    eng = nc.sync if s == "l" else nc.scalar
                    wt_sb[s] = []
                    for k in range(KT):
                        t = wtp.tile([128, 3 * H], FP8, tag=f"wt{s}{k}")
                        eng.dma_start(t[:], wt_d[s].ap()[128 * k:128 * (k + 1), :])
                        wt_sb[s].append(t)
                nc.sync.dma_start(wo_sb[:], woT_d.ap())
                nc.sync.dma_start(bo_sb[:], bo_d.ap())

                n = Bl
                for lvl in range(DEPTH):
                    last = lvl == DEPTH - 1
                    nxt = stp.tile([128, KT, 2 * n], AGDT if last else BF16,
                                   tag="st8" if last else "st",
                                   name=f"nxt{lvl}", bufs=1 if last else None)
                    for si, s in enumerate("lr"):
                        for sl in range(NSL):
                            ko0 = sl * SG
                            ps = {}
                            for gi, mb in (("r", ko0), ("z", KT + ko0), ("g", 2 * KT + ko0)):
                                p = pst.tile([128, SG, n], F32, tag="ps")
                                for mj in range(SG):
                                    m = mb + mj
                                    for k in range(KT):
                                        nc.tensor.matmul(
                                            p[:, mj, :],
                                            wt_sb[s][k][:, 128 * m:128 * (m + 1)],
                                            cur[:, k, :n],
                                            start=(k == 0), stop=(k == KT - 1))
                                ps[gi] = p
                            # biases folded in via free-dim-broadcast adds (DVE)
                            def _bias(mb_):
                                return wb_sb[s][:, mb_:mb_ + SG].unsqueeze(2)                                    .broadcast_to((128, SG, n))
                            y_r = gp.tile([128, SG, n], F32, tag="yr")
                            nc.vector.tensor_tensor(y_r[:], ps["r"][:], _bias(ko0), op=ALU.add)
                            r_t = gp.tile([128, SG, n], F32, tag="r")
                            nc.scalar.activation(r_t[:], y_r[:], AF.Sigmoid)
                            y_z = gp.tile([128, SG, n], F32, tag="yz")
                            nc.vector.tensor_tensor(y_z[:], ps["z"][:], _bias(KT + ko0), op=ALU.add)
                            z_t = gp.tile([128, SG, n], F32, tag="z")
                            nc.scalar.activation(z_t[:], y_z[:], AF.Sigmoid)
                            y_g = gp.tile([128, SG, n], F32, tag="yg")
                            nc.vector.tensor_tensor(y_g[:], ps["g"][:], _bias(2 * KT + ko0), op=ALU.add)
                            t_t = gp.tile([128, SG, n], F32, tag="t")
                            nc.vector.tensor_tensor(t_t[:], y_g[:], r_t[:], op=ALU.mult)
                            nc.vector.tensor_tensor(
                                t_t[:], t_t[:],
                                bih2_sb[s][:, ko0:ko0 + SG].unsqueeze(2)
                                .broadcast_to((128, SG, n)), op=ALU.add)
                            n_t = gp.tile([128, SG, n], F32, tag="n")
                            nc.scalar.activation(n_t[:], t_t[:], AF.Tanh)
                            u_t = gp.tile([128, SG, n], F32, tag="u")
                            nc.vector.scalar_tensor_tensor(
                                u_t[:], n_t[:], -1.0, cur[:, ko0:ko0 + SG, :n],
                                op0=ALU.mult, op1=ALU.add)  # u = h - n
                            nc.vector.tensor_tensor(u_t[:], u_t[:], z_t[:], op=ALU.mult)
                            nc.vector.tensor_tensor(
                                nxt[:, ko0:ko0 + SG, si * n:si * n + n],
                                u_t[:], n_t[:], op=ALU.add)
                    cur = nxt
                    n *= 2

                for k in range(KT):
                    eng = nc.sync if k % 2 == 0 else nc.scalar
                    eng.dma_start(leaves_bounce[128 * k:128 * (k + 1), :],
                                  cur[:, k, :])

        # ---------------- leaves all-gather ----------------
        with nc.named_scope("ag_leaves"):
            nc.gpsimd.collective_compute(
                "AllGather", ALU.bypass, replica_groups=rg,
                ins=[leaves_bounce.opt()], outs=[ag_leaves.ap()])

        # ---------------- projection + log-softmax ----------------
        with nc.named_scope("proj"):
            with tc.tile_pool(name="leaves", bufs=1) as lvp, \
                 tc.tile_pool(name="logits", bufs=3) as lgp, \
                 tc.tile_pool(name="scr", bufs=4) as scp, \
                 tc.tile_pool(name="stats", bufs=2) as sp2, \
                 tc.tile_pool(name="psproj", bufs=8, space="PSUM") as psp:
                ag_view = ag_leaves.ap().rearrange("(c h) j -> h c j", c=N_CORES)
                lvbig = lvp.tile([128, KT, N_CORES * NLOC], AGDT, tag="lvbig")
                for k in range(KT):
                    eng = nc.sync if k % 2 == 0 else nc.scalar
                    eng.dma_start(
                        lvbig[:, k, :].rearrange("p (c j) -> p c j", c=N_CORES),
                        ag_view[128 * k:128 * (k + 1)])

                # unnormalized logits stream out as soon as each row tile is
                # done; the per-shard softmax denominators are returned as a
                # tiny second output and log(sum) is folded into the host-side
                # unshard pass.
                s_all = sp2.tile([128, RT], F32, tag="sall", name="s_all")
                for r in range(RT):
                    lg = lgp.tile([128, Vs], F32, tag="lg", name=f"lg{r}")
                    sp = sp2.tile([128, NCH], F32, tag="spart", name=f"sp{r}")
                    # k-outer so one stationary (leaves) tile serves all NCH
                    # chunks; each chunk accumulates in its own PSUM bank
                    pps = [psp.tile([128, CW], F32, tag="pp", name=f"pp{r}_{nch}")
                           for nch in range(NCH)]
                    for k2 in range(KT2):
                        lhsT = lvbig[:, 2 * k2:2 * k2 + 2, 128 * r:128 * (r + 1)]
                        for nch in range(NCH):
                            nc.tensor.matmul(
                                pps[nch][:], lhsT,
                                wo_sb[:, k2, :, CW * nch:CW * (nch + 1)],
                                perf_mode=mybir.MatmulPerfMode.DoubleRow,
                                start=(k2 == 0), stop=(k2 == KT2 - 1))
                    for nch in range(NCH):
                        # bias add fused into the PSUM->SBUF copy
                        nc.vector.tensor_tensor(
                            lg[:, CW * nch:CW * (nch + 1)], pps[nch][:],
                            bo_sb[:, CW * nch:CW * (nch + 1)],
                            op=ALU.add)
                        ex = scp.tile([128, CW], BF16, tag="exp",
                                      name=f"ex{r}_{nch}")
                        nc.scalar.activation(ex[:],
                                             lg[:, CW * nch:CW * (nch + 1)],
                                             AF.Exp,
                                             accum_out=sp[:, nch:nch + 1])
                    nc.vector.reduce_sum(s_all[:, r:r + 1], sp[:],
                                         axis=mybir.AxisListType.X)
                    nc.sync.dma_start(out_d.ap()[128 * r:128 * (r + 1), :], lg[:])
                nc.scalar.dma_start(s_out_d.ap()[:, :], s_all[:])

    nc.compile()
    return nc


_CACHE = {}


def _get(B, H, V, DEPTH):
    key = (B, H, V, DEPTH)
    if key not in _CACHE:
        _CACHE[key] = _build(B, H, V, DEPTH)
    return _CACHE[key]


def _pack_inputs(B, H, V, DEPTH, encoding, Whh_l, bih_l, bhh_l, Whh_r, bih_r,
                 bhh_r, W_out, b_out):
    """Host-side shard + transpose + cast. Returns in_maps for the 8 cores."""
    KT = H // 128
    Bl = B // N_CORES
    Vs = V // N_CORES

    KT2 = KT // 2
    woT = np.ascontiguousarray(W_out.T).astype(np.float32)    # [H, V]
    encT = np.ascontiguousarray(encoding.T).astype(BFNP)      # [H, B]

    shared = {}
    for s, Whh, bih, bhh in (("l", Whh_l, bih_l, bhh_l), ("r", Whh_r, bih_r, bhh_r)):
        shared[f"wt_{s}"] = np.ascontiguousarray(Whh.T).astype(
            mybir.dt.np(FP8))  # [H, 3H] fp8: weight-load bound, not precision bound
        # bias row folded into the matmul: sigmoid gates get bih+bhh,
        # candidate gate gets bhh only (bih_n is added after the r-multiply)
        wb = np.concatenate([(bih + bhh)[:2 * H], bhh[2 * H:]])
        shared[f"wb_{s}"] = np.ascontiguousarray(
            wb.reshape(3 * KT, 128).T.astype(np.float32))
        shared[f"bih2_{s}"] = np.ascontiguousarray(
            bih[2 * H:].reshape(KT, 128).T.astype(np.float32))  # [128, KT]

    in_maps = []
    for c in range(N_CORES):
        m = dict(shared)
        m["encT"] = np.ascontiguousarray(encT[:, c * Bl:(c + 1) * Bl])
        w = woT[:, c * Vs:(c + 1) * Vs].reshape(KT2, 2, 128, Vs)
        m["woT"] = np.ascontiguousarray(
            w.transpose(2, 0, 1, 3)).astype(mybir.dt.np(FP8))
        m["bo"] = np.ascontiguousarray(np.broadcast_to(
            b_out[c * Vs:(c + 1) * Vs].astype(np.float32), (128, Vs)))
        in_maps.append(m)
    return in_maps


def _run(B, H, V, DEPTH, inputs, trace=False, nc=None):
    if nc is None:
        nc = _get(B, H, V, DEPTH)
    in_maps = _pack_inputs(B, H, V, DEPTH, **inputs)
    res = bass_utils.run_bass_kernel_spmd(
        nc, in_maps, core_ids=list(range(N_CORES)), trace=trace)

    L = 1 << DEPTH
    Bl = B // N_CORES
    Vs = V // N_CORES
    # leaf column order per core: col = jj*Bl + e with jj = bitrev(true leaf)
    rev = np.array([int(format(t, f"0{DEPTH}b")[::-1], 2) for t in range(L)])
    # log-softmax denominator: sum the per-shard exp-sums across cores
    s_tot = np.zeros((B * L,), np.float64)
    for c in range(N_CORES):
        s = res.results[c]["s_out"]                  # [128, RT]
        s_tot += s.T.reshape(-1).astype(np.float64)  # row = rt*128 + p
    lse = np.log(s_tot).astype(np.float32)           # [B*L] in device row order
    lse = lse.reshape(N_CORES, L, Bl).transpose(0, 2, 1).reshape(B, L)[:, rev]
    full = np.empty((B, L, V), np.float32)
    for c in range(N_CORES):
        o = res.results[c]["out"]                    # [B*L, Vs]
        o = o.reshape(N_CORES, L, Bl, Vs)            # [src_core, jj, e, v]
        o = o.transpose(0, 2, 1, 3).reshape(B, L, Vs)
        full[:, :, c * Vs:(c + 1) * Vs] = o[:, rev, :] - lse[:, :, None]
    return full, res


def kernel(**inputs):
    enc = np.asarray(inputs["encoding"], np.float32)
    B, H = enc.shape
    V = np.asarray(inputs["W_out"]).shape[0]
    DEPTH = int(inputs["depth"])
    args = {k: np.asarray(v, np.float32) for k, v in inputs.items() if k != "depth"}
    full, _ = _run(B, H, V, DEPTH, args)
    return full
